# revision 39
# baseline (speedup 1.0000x reference)
"""Multi-head attention (B=8, N=1024, C=1024, H=16) on 8 Trainium2 NeuronCores.

Sharding: pure data-parallel — one batch element per core, weights replicated,
no collectives.

v3 design (vs v2): startup DMA-issue fixes (per-pair packed wqk = 1 DMA/pair,
single 3D-AP DMAs for wv/wp, ones columns via memset instead of scatter DMA,
weight loads issued from the scalar-engine HWDGE queue so the sync queue only
carries x/wqk), pipeline reordered so pair-0 S+exp runs before the v-projection
(covers the wv DMA window), and the softmax-denominator broadcast done with
gpsimd partition_broadcast + DVE multiply instead of a DRAM bounce.

Per-core algorithm:
  v-proj:    v[m, dv] natural layout, interleaved [m, 16*(64+1)] with a ones
             column per head (PV then emits softmax denominators for free).
  qk-proj:   per pair p: qp[c(2 heads), n], kp[c, m] bf16 tiles in SBUF.
  attention: per (pair, nt half):
               S^T chunks for heads A,B into [128,1024] psum pairs,
               exp (ACT, 1024-wide, scale=1/8) -> eA/eB bf16 [128, 4096]
               PV: U_aug[65, nt] = v_aug.T @ expS accumulated over 8 m-chunks
             U -> SBUF (fp32, frees psum), den row 64 -> reciprocal ->
             partition_broadcast -> DVE multiply into attn_outT[c, n] bf16.
  out-proj:  out[n, d] = attn_outT.T @ wpT + bias, fp32 out.
"""

import sys

if "/opt/trn_rl_repo" not in sys.path:
    sys.path.insert(0, "/opt/trn_rl_repo")

from contextlib import ExitStack

import numpy as np

import concourse.bass as bass
import concourse.mybir as mybir
from concourse import bacc
import concourse.tile as tile
from concourse import bass_utils

B, N, C, H = 8, 1024, 1024, 16
HD = C // H          # 64
SCALE = HD ** -0.5   # 0.125
P = 128              # SBUF partitions
NT = 512             # moving-dim tile (fp32 PSUM bank limit)
NCH = C // P         # 8 contraction chunks over channels
NMT = N // P         # 8 token tiles of 128
NNT = N // NT        # 2 token tiles of 512
NPAIR = H // 2       # 8 head pairs
F32 = mybir.dt.float32
BF16 = mybir.dt.bfloat16
EXP = mybir.ActivationFunctionType.Exp


def _wait_key(w):
    return (w.sync_type, w.id, w.wait_mode, w.wait_value)


def _weights_sig(ldw):
    a = ldw.ins[0]
    return (a.memref, a.offset, tuple(tuple(x) for x in a.ap), str(a.dtype))


def _optimize_pe_stream(nc):
    """Post-compile peephole pass over the PE instruction stream.

    Operates on the PE-only subsequence (other engines' instructions are
    interleaved in the block list but the PE sequencer only sees its own
    stream; cross-engine ordering is carried entirely by semaphores).

    Rule 1 (dedupe): a wait-free LDWEIGHTS reloading the weights already in
      the array (and already consumed by a matmul) is deleted. Wait-carrying
      LDWs are kept: waits only function on LDWEIGHTS (the PE hw-decoder
      ignores waits on MATMUL), and the first load of a compiler
      [LDW, LDW, MM, MM] prefetch pair may run before the DMA-complete wait.
    Rule 2 (hoist): [LDW_A, MM_A@(0,0) K=64, LDW_B, MM_B@(64,0) K=64] ->
      [LDW_A, LDW_B, MM_A, MM_B] so the two matmuls execute concurrently on
      disjoint row groups; only when LDW_B's waits are implied by LDW_A's
      (same semaphore, same-or-lower threshold), so the earlier wait position
      cannot deadlock.
    """
    import concourse.mybir as mybir
    from collections import deque

    n_dedupe = n_hoist = 0
    for f in nc.m.functions:
        for blk in f.blocks:
            insts = blk.instructions
            pe = [a for a in insts if getattr(a, "engine", None) == mybir.EngineType.PE]
            out = []
            i = 0
            n = len(pe)
            cur_sig = None
            cur_consumed = False
            while i < n:
                a = pe[i]
                if isinstance(a, mybir.InstLdweights):
                    sig = _weights_sig(a)
                    if (
                        sig == cur_sig
                        and cur_consumed
                        and (
                            a.sync_info is None
                            or not (a.sync_info.on_wait or a.sync_info.on_update)
                        )
                        and i + 1 < n
                        and isinstance(pe[i + 1], mybir.InstMatmult)
                    ):
                        out.append(pe[i + 1])
                        i += 2
                        n_dedupe += 1
                        cur_consumed = True
                        continue
                    if (
                        i + 3 < n
                        and isinstance(pe[i + 1], mybir.InstMatmult)
                        and isinstance(pe[i + 2], mybir.InstLdweights)
                        and isinstance(pe[i + 3], mybir.InstMatmult)
                    ):
                        ldw1, mm1, ldw2, mm2 = pe[i : i + 4]
                        tp1 = mm1.tile_position
                        tp2 = mm2.tile_position
                        if (
                            tp1 is not None
                            and tp2 is not None
                            and tuple(tp1) == (0, 0)
                            and tuple(tp2) == (64, 0)
                            and ldw1.ins[0].ap[0][1] == 64
                            and ldw2.ins[0].ap[0][1] == 64
                        ):
                            w1 = [
                                _wait_key(w)
                                for w in (
                                    ldw1.sync_info.on_wait if ldw1.sync_info else []
                                )
                            ]
                            w2 = [
                                _wait_key(w)
                                for w in (
                                    ldw2.sync_info.on_wait if ldw2.sync_info else []
                                )
                            ]
                            implied = all(
                                any(
                                    k[0] == kk[0]
                                    and k[1] == kk[1]
                                    and k[2] == kk[2]
                                    and k[3] <= kk[3]
                                    for kk in w1
                                )
                                for k in w2
                            )
                            if implied and not (
                                ldw2.sync_info and ldw2.sync_info.on_update
                            ):
                                out.extend([ldw1, ldw2, mm1, mm2])
                                cur_sig = _weights_sig(ldw2)
                                cur_consumed = True
                                i += 4
                                n_hoist += 1
                                continue
                    cur_sig = sig
                    cur_consumed = False
                elif isinstance(a, mybir.InstMatmult):
                    cur_consumed = True
                else:
                    cur_sig = None  # unknown PE instruction: be conservative
                    cur_consumed = False
                out.append(a)
                i += 1
            # weave the transformed PE stream back into the block, keeping
            # non-PE instructions in place; trailing PE slots left over from
            # deletions are simply skipped.
            pe_q = deque(out)
            new_insts = []
            for a in insts:
                if getattr(a, "engine", None) == mybir.EngineType.PE:
                    if pe_q:
                        new_insts.append(pe_q.popleft())
                else:
                    new_insts.append(a)
            assert not pe_q, "transformed PE stream longer than original slots"
            blk.instructions = new_insts
    print(f"_optimize_pe_stream: {n_dedupe} LDW deduped, {n_hoist} LDW hoisted")


def build_module():
    nc = bacc.Bacc("TRN2", target_bir_lowering=False, debug=False, num_devices=B)

    xT = nc.dram_tensor("xT", [C, N], BF16, kind="ExternalInput").ap()
    # per-pair packed qk weights: row p*128+r, col ck*256 + (q j | k j)
    wqkP = nc.dram_tensor("wqkP", [NPAIR * P, NCH * 2 * P], BF16, kind="ExternalInput").ap()
    wvT = nc.dram_tensor("wvT", [C, C], BF16, kind="ExternalInput").ap()
    wpT = nc.dram_tensor("wpT", [C, C], BF16, kind="ExternalInput").ap()
    bias = nc.dram_tensor("bias_bc", [P, C], F32, kind="ExternalInput").ap()
    out = nc.dram_tensor("out", [N, C], F32, kind="ExternalOutput").ap()

    with tile.TileContext(nc) as tc, ExitStack() as ctx:
        dram = ctx.enter_context(tc.tile_pool(name="dram", bufs=1, space="DRAM"))
        rden_d = dram.tile([H * NNT, NT], F32, tag="rden_d", name="rden_d")

        xt_pool = ctx.enter_context(tc.tile_pool(name="xt", bufs=8))
        qk_pool = ctx.enter_context(tc.tile_pool(name="qk", bufs=7))
        v_pool = ctx.enter_context(tc.tile_pool(name="v", bufs=8))
        e_pool = ctx.enter_context(tc.tile_pool(name="e", bufs=6))
        u_pool = ctx.enter_context(tc.tile_pool(name="u", bufs=6))
        aot_pool = ctx.enter_context(tc.tile_pool(name="aot", bufs=1))
        w_pool = ctx.enter_context(tc.tile_pool(name="wst", bufs=2))
        wqk_pool = ctx.enter_context(tc.tile_pool(name="wqk", bufs=4))
        den_pool = ctx.enter_context(tc.tile_pool(name="den", bufs=4))
        rbc_pool = ctx.enter_context(tc.tile_pool(name="rbc", bufs=4))
        one_pool = ctx.enter_context(tc.tile_pool(name="one", bufs=1))
        stage_pool = ctx.enter_context(tc.tile_pool(name="stage", bufs=2))
        s_psum = ctx.enter_context(tc.tile_pool(name="s_ps", bufs=2, space="PSUM"))
        pv_psum = ctx.enter_context(tc.tile_pool(name="pv_ps", bufs=2, space="PSUM"))
        pj_psum = ctx.enter_context(tc.tile_pool(name="pj_ps", bufs=1, space="PSUM"))

        # ---------- qk weight load (packed per pair; 2 dmas each) ----------
        wqk_tiles = {}

        def load_wqk(p, eng=None):
            eng = eng or nc.sync
            w_t = wqk_pool.tile([P, NCH * 2 * P], BF16, tag="wqk", name=f"wqk{p}")
            # q half then k half so the first q matmul doesn't wait for k
            eng.dma_start(w_t[:, 0:C], wqkP[p * P : (p + 1) * P, 0:C])
            eng.dma_start(w_t[:, C : 2 * C], wqkP[p * P : (p + 1) * P, C : 2 * C])
            wqk_tiles[p] = w_t

        # ---------- input loads: x on sync queue, weights on scalar queue ----
        # few, large dma_starts: the DMA-semaphore ring only allows ~8-10
        # outstanding dma_starts, so a long run of small loads stalls the
        # issue queue for tens of us.
        # Startup loads: ~8-9 DMA sems can be outstanding, so order matters.
        # sync queue: wqk0-q, xt0-3, wqk0-k, xt4-7 (first matmul deps first);
        # scalar queue: wv halves, then wqk1 (bias/wp fill in later).
        w0 = wqk_pool.tile([P, NCH * 2 * P], BF16, tag="wqk", name="wqk0")
        xts = [xt_pool.tile([P, N], BF16, tag="xt", name=f"xt{t}") for t in range(NCH)]
        nc.sync.dma_start(w0[:, 0:C], wqkP[0:P, 0:C])
        for t in range(0, 4):
            nc.sync.dma_start(xts[t], xT[t * P : (t + 1) * P, :])
        nc.sync.dma_start(w0[:, C : 2 * C], wqkP[0:P, C : 2 * C])
        for t in range(4, NCH):
            nc.sync.dma_start(xts[t], xT[t * P : (t + 1) * P, :])
        wqk_tiles[0] = w0

        # wv as one [128, 8*1024] tile: block ck at cols [ck*1024, +1024)
        wv_sb = w_pool.tile([P, NCH * C], BF16, tag="wst", name="wv_sb")
        for h in range(2):
            wv_src = bass.AP(
                tensor=wvT.tensor,
                offset=h * 4 * P * C,
                ap=[[C, P], [P * C, 4], [1, C]],
            )
            nc.scalar.dma_start(wv_sb[:, h * 4 * C : (h + 1) * 4 * C], wv_src)
        load_wqk(1, eng=nc.scalar)

        vsb = []
        for mt in range(NMT):
            v_t = v_pool.tile([P, H * (HD + 1)], BF16, tag="v", name=f"v{mt}")
            # contiguous full-tile fill; the v-proj copy then overwrites the
            # value columns, leaving 1.0 in each head's 65th (ones) column
            nc.gpsimd.memset(v_t, 1.0)
            vsb.append(v_t)
        aot = [
            aot_pool.tile([P, N], BF16, tag=f"aot{t}", name=f"aot{t}")
            for t in range(NCH)
        ]

        # ---------- v projection (natural layout + ones cols) ----------
        def emit_vblock(mt):
                ps = pj_psum.tile([P, 2 * NT], F32, tag="pj", name=f"psv{mt}")
                for ck in range(NCH):
                    for dvt in range(NNT):
                        nc.tensor.matmul(
                            ps[:, dvt * NT : (dvt + 1) * NT],
                            lhsT=xts[ck][:, mt * P : (mt + 1) * P],
                            rhs=wv_sb[:, ck * C + dvt * NT : ck * C + (dvt + 1) * NT],
                            start=(ck == 0),
                            stop=(ck == NCH - 1),
                        )
                for dvt in range(NNT):
                    nc.vector.tensor_copy(
                        vsb[mt].rearrange("p (h w) -> p h w", w=HD + 1)[
                            :, dvt * NCH : (dvt + 1) * NCH, 0:HD
                        ],
                        ps[:, dvt * NT : (dvt + 1) * NT].rearrange(
                            "p (h w) -> p h w", w=HD
                        ),
                    )

        # ---------- qk projection for one head pair, SBUF-resident ----------
        def emit_qkproj(p):
            qp = qk_pool.tile([P, N], BF16, tag="qk", name=f"qp{p}")
            kp = qk_pool.tile([P, N], BF16, tag="qk", name=f"kp{p}")
            w_t = wqk_tiles.pop(p)
            for which, dstt in ((0, qp), (1, kp)):
                ps = pj_psum.tile([P, 2 * NT], F32, tag="pj", name=f"psqk{which}_{p}")
                for ck in range(NCH):
                    for nt_ in range(NNT):
                        nc.tensor.matmul(
                            ps[:, nt_ * NT : (nt_ + 1) * NT],
                            lhsT=w_t[:, which * C + ck * P : which * C + (ck + 1) * P],
                            rhs=xts[ck][:, nt_ * NT : (nt_ + 1) * NT],
                            start=(ck == 0),
                            stop=(ck == NCH - 1),
                        )
                # drain the two banks on DVE and ACT concurrently to halve
                # the pj-psum hold time (pj pool is single-buffered)
                nc.vector.tensor_copy(dstt[:, 0:NT], ps[:, 0:NT])
                nc.scalar.copy(dstt[:, NT : 2 * NT], ps[:, NT : 2 * NT])
            return qp, kp

        # ---------- attention ----------
        pair_units = {}

        def emit_denorm(p, nt_, punits):
            """den rows -> reciprocal -> DRAM bounce -> partition-broadcast
            read -> DVE multiply into the attn-out tiles (bf16 cast on write).
            Runs per (pair, nt-half) so the out-proj's first n-half unblocks
            as soon as the last pair's nt=0 units are normalized."""
            g = p * 4 + nt_ * 2
            den_g = den_pool.tile([2, NT], F32, tag="den", name=f"den{p}_{nt_}")
            for i, (h, u_t) in enumerate(punits):
                nc.sync.dma_start(den_g[i : i + 1, :], u_t[HD : HD + 1, :])
            rden = den_pool.tile([2, NT], F32, tag="rden", name=f"rden{p}_{nt_}")
            nc.vector.reciprocal_approx_fast(out=rden, in_=den_g)
            nc.sync.dma_start(rden_d[g : g + 2, :], rden)
            for i, (h, u_t) in enumerate(punits):
                rbc = rbc_pool.tile([HD, NT], F32, tag="rbc", name=f"rbc{h}_{nt_}")
                src_ = rden_d[g + i : g + i + 1, :]
                bsrc = bass.AP(
                    tensor=src_.tensor,
                    offset=src_.offset,
                    ap=[[0, HD], list(src_.ap[-1])],
                )
                nc.sync.dma_start(out=rbc, in_=bsrc)
                ct, prow = h // 2, (h % 2) * HD
                nc.vector.tensor_mul(
                    aot[ct][prow : prow + HD, nt_ * NT : (nt_ + 1) * NT],
                    u_t[0:HD, :],
                    rbc,
                )

        def emit_s_exp_nt(p, nt_, qp, kp):
            """S^T + exp for one (pair, n-half). Heads A and B share one
            [128, 1024] psum tile per m-chunk (A in the low bank, B in the
            high bank) so both matmuls become ready together; the post-compile
            pass hoists B's LDWEIGHTS above A's matmul, making the two K=64
            matmuls (array rows 0-63 / 64-127) run concurrently.
            Returns two e tiles [128, 4096] (mc 0-3 and mc 4-7), each laid
            out as [A_mc|B_mc|...]; the split lets PV release the first half
            mid-chain so the next pair's S can reuse the slots earlier."""
            eA = e_pool.tile([P, NMT * NT], BF16, tag="e", name=f"e{p}_{nt_}a")
            eB = e_pool.tile([P, NMT * NT], BF16, tag="e", name=f"e{p}_{nt_}b")
            for mc in range(NMT):
                e_t = eA if mc < 4 else eB
                s_t = s_psum.tile([P, 2 * NT], F32, tag="s", name=f"s{p}_{nt_}_{mc}")
                # high priority: the S pair feeds ACT (the attention-phase
                # pacer) and must pop back-to-back so the post-compile hoist
                # can make the two K=64 row-tiles run concurrently.
                with tc.high_priority():
                    nc.tensor.matmul(
                        s_t[:, 0:NT],
                        lhsT=kp[0:HD, mc * P : (mc + 1) * P],
                        rhs=qp[0:HD, nt_ * NT : (nt_ + 1) * NT],
                        start=True,
                        stop=True,
                    )
                    nc.tensor.matmul(
                        s_t[:, NT : 2 * NT],
                        lhsT=kp[HD:P, mc * P : (mc + 1) * P],
                        rhs=qp[HD:P, nt_ * NT : (nt_ + 1) * NT],
                        start=True,
                        stop=True,
                    )
                nc.scalar.activation(
                    e_t[:, (mc % 4) * 2 * NT : ((mc % 4) + 1) * 2 * NT],
                    s_t,
                    EXP,
                    scale=SCALE,
                )
            return eA, eB

        def emit_pv_nt(p, nt_, e_h):
            """PV for BOTH heads of the pair over one n-half, the two chains
            interleaved per m-chunk in the two pv psum slots. Both heads pass
            mc 0-3 together, so the first e-half frees as early as possible
            for the next pair's S chain."""
            eA, eB = e_h
            hA, hB = 2 * p, 2 * p + 1
            psA = pv_psum.tile([HD + 1, NT], F32, tag="pv", name=f"pu{hA}_{nt_}")
            psB = pv_psum.tile([HD + 1, NT], F32, tag="pv", name=f"pu{hB}_{nt_}")
            for mc in range(NMT):
                e_t = eA if mc < 4 else eB
                for ps, j in ((psA, 0), (psB, 1)):
                    nc.tensor.matmul(
                        ps,
                        lhsT=vsb[mc][:, (2 * p + j) * (HD + 1) : (2 * p + j + 1) * (HD + 1)],
                        rhs=e_t[:, ((mc % 4) * 2 + j) * NT : ((mc % 4) * 2 + j + 1) * NT],
                        start=(mc == 0),
                        stop=(mc == NMT - 1),
                    )
            units = []
            for h, ps in ((hA, psA), (hB, psB)):
                u_t = u_pool.tile([HD + 1, NT], F32, tag="u", name=f"u{h}_{nt_}")
                nc.vector.tensor_copy(u_t, ps)
                units.append((h, u_t))
            emit_denorm(p, nt_, units)

        # ---------- output projection + bias ----------
        # dt halves paired on the stationary aot chunk, single [128, 1024]
        # psum per n-tile; alternate between the pj and s psum pools (the s
        # pool is free by the tail) to keep the tail double-buffered.
        wp_holder = []
        bias_holder = []

        def load_wp():
            bias_sb = one_pool.tile([P, C], F32, tag="bias", name="bias_sb")
            nc.scalar.dma_start(bias_sb, bias)
            bias_holder.append(bias_sb)
            wp_sb = w_pool.tile([P, NCH * C], BF16, tag="wst", name="wp_sb")
            wp_src = bass.AP(
                tensor=wpT.tensor,
                offset=0,
                ap=[[C, P], [P * C, NCH], [1, C]],
            )
            nc.scalar.dma_start(wp_sb, wp_src)
            wp_holder.append(wp_sb)

        def emit_outproj():
            bias_sb = bias_holder[0]
            wp_sb = wp_holder[0]
            for nt2 in range(NMT):
                pool = pj_psum if nt2 % 2 == 0 else s_psum
                ps = pool.tile(
                    [P, 2 * NT], F32, tag="pj" if nt2 % 2 == 0 else "s",
                    name=f"pso{nt2}",
                )
                o_sb = stage_pool.tile([P, 2 * NT], F32, tag="stage", name=f"o{nt2}")
                for ck in range(NCH):
                    for dt in range(NNT):
                        nc.tensor.matmul(
                            ps[:, dt * NT : (dt + 1) * NT],
                            lhsT=aot[ck][:, nt2 * P : (nt2 + 1) * P],
                            rhs=wp_sb[:, ck * C + dt * NT : ck * C + (dt + 1) * NT],
                            start=(ck == 0),
                            stop=(ck == NCH - 1),
                        )
                # finer drain granularity on the last tile shortens the
                # add->DMA tail after the final matmul
                nq = 4 if nt2 == NMT - 1 else 2
                qw = 2 * NT // nq
                for dq in range(nq):
                    nc.vector.tensor_add(
                        o_sb[:, dq * qw : (dq + 1) * qw],
                        ps[:, dq * qw : (dq + 1) * qw],
                        bias_sb[:, dq * qw : (dq + 1) * qw],
                    )
                    nc.sync.dma_start(
                        out[nt2 * P : (nt2 + 1) * P, dq * qw : (dq + 1) * qw],
                        o_sb[:, dq * qw : (dq + 1) * qw],
                    )

        # ---------- pipeline ----------
        # pair-0 qk-proj + pair-0 S/exp run while x/wqk1/wv stream in;
        # v-blocks follow, then PV(0) consumes them and the steady loop runs.
        qks = {}
        qp0, kp0 = emit_qkproj(0)
        e00 = emit_s_exp_nt(0, 0, qp0, kp0)
        e01 = emit_s_exp_nt(0, 1, qp0, kp0)
        qks[1] = emit_qkproj(1)
        load_wqk(2)
        for mt in range(NMT):
            emit_vblock(mt)
        emit_pv_nt(0, 0, e00)
        emit_pv_nt(0, 1, e01)
        qks[2] = emit_qkproj(2)
        for p in range(1, NPAIR):
            qp, kp = qks.pop(p)
            e0 = emit_s_exp_nt(p, 0, qp, kp)
            emit_pv_nt(p, 0, e0)
            e1 = emit_s_exp_nt(p, 1, qp, kp)
            emit_pv_nt(p, 1, e1)
            if p + 2 < NPAIR:
                load_wqk(p + 2)
                qks[p + 2] = emit_qkproj(p + 2)
            if p == 6:
                load_wp()
        emit_outproj()

    nc.compile()
    _optimize_pe_stream(nc)
    return nc


def make_in_maps(x, w_qkv, w_proj, b_proj):
    import ml_dtypes

    bf16 = ml_dtypes.bfloat16
    # packed per-pair qk weights: wqkP[p*128+r, ck*128 + j]       = Wq[p,j,ck,r]
    #                             wqkP[p*128+r, C + ck*128 + j]   = Wk[p,j,ck,r]
    Wq = w_qkv[:C].reshape(NPAIR, P, NCH, P)        # [p, j, ck, r]
    Wk = w_qkv[C : 2 * C].reshape(NPAIR, P, NCH, P)
    Aq = Wq.transpose(0, 3, 2, 1)                   # [p, r, ck, j]
    Ak = Wk.transpose(0, 3, 2, 1)
    wqkP = np.ascontiguousarray(
        np.concatenate([Aq.reshape(NPAIR, P, C), Ak.reshape(NPAIR, P, C)], axis=2)
        .reshape(NPAIR * P, 2 * C)
        .astype(bf16)
    )
    wvT = np.ascontiguousarray(w_qkv[2 * C :].T.astype(bf16))
    wpT = np.ascontiguousarray(w_proj.T.astype(bf16))
    bias_bc = np.ascontiguousarray(
        np.broadcast_to(b_proj, (P, C)).astype(np.float32)
    )
    in_maps = []
    for b in range(B):
        in_maps.append(
            {
                "xT": np.ascontiguousarray(x[b].T.astype(bf16)),
                "wqkP": wqkP,
                "wvT": wvT,
                "wpT": wpT,
                "bias_bc": bias_bc,
            }
        )
    return in_maps


_CACHED_NC = None


def kernel(x, w_qkv, w_proj, b_proj):
    global _CACHED_NC
    x = np.asarray(x, dtype=np.float32)
    w_qkv = np.asarray(w_qkv, dtype=np.float32)
    w_proj = np.asarray(w_proj, dtype=np.float32)
    b_proj = np.asarray(b_proj, dtype=np.float32)
    if _CACHED_NC is None:
        _CACHED_NC = build_module()
    nc = _CACHED_NC
    in_maps = make_in_maps(x, w_qkv, w_proj, b_proj)
    res = bass_utils.run_bass_kernel_spmd(nc, in_maps, core_ids=list(range(B)))
    return np.stack([res.results[b]["out"] for b in range(B)], axis=0)


if __name__ == "__main__":
    nc = build_module()
    ninst = sum(len(b.instructions) for b in nc.m.functions[0].blocks)
    print("module built ok;", ninst, "instructions")


# revision 40
# speedup vs baseline: 1.1763x; 1.1763x over previous
"""Multi-head attention (B=8, N=1024, C=1024, H=16) on 8 Trainium2 NeuronCores.

Sharding: pure data-parallel — one batch element per core, weights replicated,
no collectives.

v3 design (vs v2): startup DMA-issue fixes (per-pair packed wqk = 1 DMA/pair,
single 3D-AP DMAs for wv/wp, ones columns via memset instead of scatter DMA,
weight loads issued from the scalar-engine HWDGE queue so the sync queue only
carries x/wqk), pipeline reordered so pair-0 S+exp runs before the v-projection
(covers the wv DMA window), and the softmax-denominator broadcast done with
gpsimd partition_broadcast + DVE multiply instead of a DRAM bounce.

Per-core algorithm:
  v-proj:    v[m, dv] natural layout, interleaved [m, 16*(64+1)] with a ones
             column per head (PV then emits softmax denominators for free).
  qk-proj:   per pair p: qp[c(2 heads), n], kp[c, m] bf16 tiles in SBUF.
  attention: per (pair, nt half):
               S^T chunks for heads A,B into [128,1024] psum pairs,
               exp (ACT, 1024-wide, scale=1/8) -> eA/eB bf16 [128, 4096]
               PV: U_aug[65, nt] = v_aug.T @ expS accumulated over 8 m-chunks
             U -> SBUF (fp32, frees psum), den row 64 -> reciprocal ->
             partition_broadcast -> DVE multiply into attn_outT[c, n] bf16.
  out-proj:  out[n, d] = attn_outT.T @ wpT + bias, fp32 out.
"""

import sys

if "/opt/trn_rl_repo" not in sys.path:
    sys.path.insert(0, "/opt/trn_rl_repo")

from contextlib import ExitStack

import numpy as np

import concourse.bass as bass
import concourse.mybir as mybir
from concourse import bacc
import concourse.tile as tile
from concourse import bass_utils

B, N, C, H = 8, 1024, 1024, 16
HD = C // H          # 64
SCALE = HD ** -0.5   # 0.125
P = 128              # SBUF partitions
NT = 512             # moving-dim tile (fp32 PSUM bank limit)
NCH = C // P         # 8 contraction chunks over channels
NMT = N // P         # 8 token tiles of 128
NNT = N // NT        # 2 token tiles of 512
NPAIR = H // 2       # 8 head pairs
F32 = mybir.dt.float32
BF16 = mybir.dt.bfloat16
EXP = mybir.ActivationFunctionType.Exp


def _wait_key(w):
    return (w.sync_type, w.id, w.wait_mode, w.wait_value)


def _weights_sig(ldw):
    a = ldw.ins[0]
    return (a.memref, a.offset, tuple(tuple(x) for x in a.ap), str(a.dtype))


def _optimize_pe_stream(nc):
    """Post-compile peephole pass over the PE instruction stream.

    Operates on the PE-only subsequence (other engines' instructions are
    interleaved in the block list but the PE sequencer only sees its own
    stream; cross-engine ordering is carried entirely by semaphores).

    Rule 1 (dedupe): a wait-free LDWEIGHTS reloading the weights already in
      the array (and already consumed by a matmul) is deleted. Wait-carrying
      LDWs are kept: waits only function on LDWEIGHTS (the PE hw-decoder
      ignores waits on MATMUL), and the first load of a compiler
      [LDW, LDW, MM, MM] prefetch pair may run before the DMA-complete wait.
    Rule 2 (hoist): [LDW_A, MM_A@(0,0) K=64, LDW_B, MM_B@(64,0) K=64] ->
      [LDW_A, LDW_B, MM_A, MM_B] so the two matmuls execute concurrently on
      disjoint row groups; only when LDW_B's waits are implied by LDW_A's
      (same semaphore, same-or-lower threshold), so the earlier wait position
      cannot deadlock.
    """
    import concourse.mybir as mybir
    from collections import deque

    n_dedupe = n_hoist = 0
    for f in nc.m.functions:
        for blk in f.blocks:
            insts = blk.instructions
            pe = [a for a in insts if getattr(a, "engine", None) == mybir.EngineType.PE]
            out = []
            i = 0
            n = len(pe)
            cur_sig = None
            cur_consumed = False
            while i < n:
                a = pe[i]
                if isinstance(a, mybir.InstLdweights):
                    sig = _weights_sig(a)
                    if (
                        sig == cur_sig
                        and cur_consumed
                        and (
                            a.sync_info is None
                            or not (a.sync_info.on_wait or a.sync_info.on_update)
                        )
                        and i + 1 < n
                        and isinstance(pe[i + 1], mybir.InstMatmult)
                    ):
                        out.append(pe[i + 1])
                        i += 2
                        n_dedupe += 1
                        cur_consumed = True
                        continue
                    if (
                        i + 3 < n
                        and isinstance(pe[i + 1], mybir.InstMatmult)
                        and isinstance(pe[i + 2], mybir.InstLdweights)
                        and isinstance(pe[i + 3], mybir.InstMatmult)
                    ):
                        ldw1, mm1, ldw2, mm2 = pe[i : i + 4]
                        tp1 = mm1.tile_position
                        tp2 = mm2.tile_position
                        if (
                            tp1 is not None
                            and tp2 is not None
                            and tuple(tp1) == (0, 0)
                            and tuple(tp2) == (64, 0)
                            and ldw1.ins[0].ap[0][1] == 64
                            and ldw2.ins[0].ap[0][1] == 64
                        ):
                            w1 = [
                                _wait_key(w)
                                for w in (
                                    ldw1.sync_info.on_wait if ldw1.sync_info else []
                                )
                            ]
                            w2 = [
                                _wait_key(w)
                                for w in (
                                    ldw2.sync_info.on_wait if ldw2.sync_info else []
                                )
                            ]
                            implied = all(
                                any(
                                    k[0] == kk[0]
                                    and k[1] == kk[1]
                                    and k[2] == kk[2]
                                    and k[3] <= kk[3]
                                    for kk in w1
                                )
                                for k in w2
                            )
                            if implied and not (
                                ldw2.sync_info and ldw2.sync_info.on_update
                            ):
                                out.extend([ldw1, ldw2, mm1, mm2])
                                cur_sig = _weights_sig(ldw2)
                                cur_consumed = True
                                i += 4
                                n_hoist += 1
                                continue
                    cur_sig = sig
                    cur_consumed = False
                elif isinstance(a, mybir.InstMatmult):
                    cur_consumed = True
                else:
                    cur_sig = None  # unknown PE instruction: be conservative
                    cur_consumed = False
                out.append(a)
                i += 1
            # weave the transformed PE stream back into the block, keeping
            # non-PE instructions in place; trailing PE slots left over from
            # deletions are simply skipped.
            pe_q = deque(out)
            new_insts = []
            for a in insts:
                if getattr(a, "engine", None) == mybir.EngineType.PE:
                    if pe_q:
                        new_insts.append(pe_q.popleft())
                else:
                    new_insts.append(a)
            assert not pe_q, "transformed PE stream longer than original slots"
            blk.instructions = new_insts
    print(f"_optimize_pe_stream: {n_dedupe} LDW deduped, {n_hoist} LDW hoisted")


def build_module():
    nc = bacc.Bacc("TRN2", target_bir_lowering=False, debug=False, num_devices=B)

    xT = nc.dram_tensor("xT", [C, N], BF16, kind="ExternalInput").ap()
    # per-pair packed qk weights: row p*128+r, col ck*256 + (q j | k j)
    wqkP = nc.dram_tensor("wqkP", [NPAIR * P, NCH * 2 * P], BF16, kind="ExternalInput").ap()
    wvT = nc.dram_tensor("wvT", [C, C], BF16, kind="ExternalInput").ap()
    wpT = nc.dram_tensor("wpT", [C, C], BF16, kind="ExternalInput").ap()
    bias = nc.dram_tensor("bias_bc", [P, C], F32, kind="ExternalInput").ap()
    out = nc.dram_tensor("out", [N, C], F32, kind="ExternalOutput").ap()

    with tile.TileContext(nc) as tc, ExitStack() as ctx:
        dram = ctx.enter_context(tc.tile_pool(name="dram", bufs=1, space="DRAM"))
        rden_d = dram.tile([H * NNT, NT], F32, tag="rden_d", name="rden_d")

        xt_pool = ctx.enter_context(tc.tile_pool(name="xt", bufs=8))
        qk_pool = ctx.enter_context(tc.tile_pool(name="qk", bufs=7))
        v_pool = ctx.enter_context(tc.tile_pool(name="v", bufs=8))
        e_pool = ctx.enter_context(tc.tile_pool(name="e", bufs=6))
        u_pool = ctx.enter_context(tc.tile_pool(name="u", bufs=6))
        aot_pool = ctx.enter_context(tc.tile_pool(name="aot", bufs=1))
        w_pool = ctx.enter_context(tc.tile_pool(name="wst", bufs=2))
        wqk_pool = ctx.enter_context(tc.tile_pool(name="wqk", bufs=4))
        den_pool = ctx.enter_context(tc.tile_pool(name="den", bufs=4))
        rbc_pool = ctx.enter_context(tc.tile_pool(name="rbc", bufs=4))
        one_pool = ctx.enter_context(tc.tile_pool(name="one", bufs=1))
        stage_pool = ctx.enter_context(tc.tile_pool(name="stage", bufs=2))
        s_psum = ctx.enter_context(tc.tile_pool(name="s_ps", bufs=2, space="PSUM"))
        pv_psum = ctx.enter_context(tc.tile_pool(name="pv_ps", bufs=2, space="PSUM"))
        pj_psum = ctx.enter_context(tc.tile_pool(name="pj_ps", bufs=1, space="PSUM"))

        # ---------- qk weight load (packed per pair; 2 dmas each) ----------
        wqk_tiles = {}

        def load_wqk(p, eng=None):
            eng = eng or nc.sync
            w_t = wqk_pool.tile([P, NCH * 2 * P], BF16, tag="wqk", name=f"wqk{p}")
            # q half then k half so the first q matmul doesn't wait for k
            eng.dma_start(w_t[:, 0:C], wqkP[p * P : (p + 1) * P, 0:C])
            eng.dma_start(w_t[:, C : 2 * C], wqkP[p * P : (p + 1) * P, C : 2 * C])
            wqk_tiles[p] = w_t

        # ---------- input loads: x on sync queue, weights on scalar queue ----
        # few, large dma_starts: the DMA-semaphore ring only allows ~8-10
        # outstanding dma_starts, so a long run of small loads stalls the
        # issue queue for tens of us.
        # Startup loads: ~8-9 DMA sems can be outstanding, so order matters.
        # sync queue: wqk0-q, xt0-3, wqk0-k, xt4-7 (first matmul deps first);
        # scalar queue: wv halves, then wqk1 (bias/wp fill in later).
        w0 = wqk_pool.tile([P, NCH * 2 * P], BF16, tag="wqk", name="wqk0")
        xts = [xt_pool.tile([P, N], BF16, tag="xt", name=f"xt{t}") for t in range(NCH)]
        nc.sync.dma_start(w0[:, 0:C], wqkP[0:P, 0:C])
        for t in range(0, 4):
            nc.sync.dma_start(xts[t], xT[t * P : (t + 1) * P, :])
        nc.sync.dma_start(w0[:, C : 2 * C], wqkP[0:P, C : 2 * C])
        for t in range(4, NCH):
            nc.sync.dma_start(xts[t], xT[t * P : (t + 1) * P, :])
        wqk_tiles[0] = w0

        # wv as one [128, 8*1024] tile: block ck at cols [ck*1024, +1024)
        wv_sb = w_pool.tile([P, NCH * C], BF16, tag="wst", name="wv_sb")
        for h in range(2):
            wv_src = bass.AP(
                tensor=wvT.tensor,
                offset=h * 4 * P * C,
                ap=[[C, P], [P * C, 4], [1, C]],
            )
            nc.scalar.dma_start(wv_sb[:, h * 4 * C : (h + 1) * 4 * C], wv_src)
        load_wqk(1, eng=nc.scalar)

        vsb = []
        for mt in range(NMT):
            v_t = v_pool.tile([P, H * (HD + 1)], BF16, tag="v", name=f"v{mt}")
            # contiguous full-tile fill; the v-proj copy then overwrites the
            # value columns, leaving 1.0 in each head's 65th (ones) column
            nc.gpsimd.memset(v_t, 1.0)
            vsb.append(v_t)
        aot = [
            aot_pool.tile([P, N], BF16, tag=f"aot{t}", name=f"aot{t}")
            for t in range(NCH)
        ]

        # ---------- v projection (natural layout + ones cols) ----------
        def emit_vblock(mt):
                ps = pj_psum.tile([P, 2 * NT], F32, tag="pj", name=f"psv{mt}")
                for ck in range(NCH):
                    for dvt in range(NNT):
                        nc.tensor.matmul(
                            ps[:, dvt * NT : (dvt + 1) * NT],
                            lhsT=xts[ck][:, mt * P : (mt + 1) * P],
                            rhs=wv_sb[:, ck * C + dvt * NT : ck * C + (dvt + 1) * NT],
                            start=(ck == 0),
                            stop=(ck == NCH - 1),
                        )
                for dvt in range(NNT):
                    nc.vector.tensor_copy(
                        vsb[mt].rearrange("p (h w) -> p h w", w=HD + 1)[
                            :, dvt * NCH : (dvt + 1) * NCH, 0:HD
                        ],
                        ps[:, dvt * NT : (dvt + 1) * NT].rearrange(
                            "p (h w) -> p h w", w=HD
                        ),
                    )

        # ---------- qk projection for one head pair, SBUF-resident ----------
        def emit_qkproj(p):
            qp = qk_pool.tile([P, N], BF16, tag="qk", name=f"qp{p}")
            kp = qk_pool.tile([P, N], BF16, tag="qk", name=f"kp{p}")
            w_t = wqk_tiles.pop(p)
            for which, dstt in ((0, qp), (1, kp)):
                ps = pj_psum.tile([P, 2 * NT], F32, tag="pj", name=f"psqk{which}_{p}")
                for ck in range(NCH):
                    for nt_ in range(NNT):
                        nc.tensor.matmul(
                            ps[:, nt_ * NT : (nt_ + 1) * NT],
                            lhsT=w_t[:, which * C + ck * P : which * C + (ck + 1) * P],
                            rhs=xts[ck][:, nt_ * NT : (nt_ + 1) * NT],
                            start=(ck == 0),
                            stop=(ck == NCH - 1),
                        )
                for nt_ in range(NNT):
                    nc.vector.tensor_copy(
                        dstt[:, nt_ * NT : (nt_ + 1) * NT],
                        ps[:, nt_ * NT : (nt_ + 1) * NT],
                    )
            return qp, kp

        # ---------- attention ----------
        pair_units = {}

        def emit_denorm(p, nt_, punits):
            """den rows -> reciprocal -> DRAM bounce -> partition-broadcast
            read -> DVE multiply into the attn-out tiles (bf16 cast on write).
            Runs per (pair, nt-half) so the out-proj's first n-half unblocks
            as soon as the last pair's nt=0 units are normalized."""
            g = p * 4 + nt_ * 2
            den_g = den_pool.tile([2, NT], F32, tag="den", name=f"den{p}_{nt_}")
            for i, (h, u_t) in enumerate(punits):
                nc.sync.dma_start(den_g[i : i + 1, :], u_t[HD : HD + 1, :])
            rden = den_pool.tile([2, NT], F32, tag="rden", name=f"rden{p}_{nt_}")
            nc.vector.reciprocal_approx_fast(out=rden, in_=den_g)
            nc.sync.dma_start(rden_d[g : g + 2, :], rden)
            for i, (h, u_t) in enumerate(punits):
                rbc = rbc_pool.tile([HD, NT], F32, tag="rbc", name=f"rbc{h}_{nt_}")
                src_ = rden_d[g + i : g + i + 1, :]
                bsrc = bass.AP(
                    tensor=src_.tensor,
                    offset=src_.offset,
                    ap=[[0, HD], list(src_.ap[-1])],
                )
                nc.sync.dma_start(out=rbc, in_=bsrc)
                ct, prow = h // 2, (h % 2) * HD
                nc.vector.tensor_mul(
                    aot[ct][prow : prow + HD, nt_ * NT : (nt_ + 1) * NT],
                    u_t[0:HD, :],
                    rbc,
                )

        def emit_s_exp_nt(p, nt_, qp, kp):
            """S^T + exp for one (pair, n-half). Heads A and B share one
            [128, 1024] psum tile per m-chunk (A in the low bank, B in the
            high bank) so both matmuls become ready together; the post-compile
            pass hoists B's LDWEIGHTS above A's matmul, making the two K=64
            matmuls (array rows 0-63 / 64-127) run concurrently.
            Returns two e tiles [128, 4096] (mc 0-3 and mc 4-7), each laid
            out as [A_mc|B_mc|...]; the split lets PV release the first half
            mid-chain so the next pair's S can reuse the slots earlier."""
            eA = e_pool.tile([P, NMT * NT], BF16, tag="e", name=f"e{p}_{nt_}a")
            eB = e_pool.tile([P, NMT * NT], BF16, tag="e", name=f"e{p}_{nt_}b")
            for mc in range(NMT):
                e_t = eA if mc < 4 else eB
                s_t = s_psum.tile([P, 2 * NT], F32, tag="s", name=f"s{p}_{nt_}_{mc}")
                # high priority: the S pair feeds ACT (the attention-phase
                # pacer) and must pop back-to-back so the post-compile hoist
                # can make the two K=64 row-tiles run concurrently.
                with tc.high_priority():
                    nc.tensor.matmul(
                        s_t[:, 0:NT],
                        lhsT=kp[0:HD, mc * P : (mc + 1) * P],
                        rhs=qp[0:HD, nt_ * NT : (nt_ + 1) * NT],
                        start=True,
                        stop=True,
                    )
                    nc.tensor.matmul(
                        s_t[:, NT : 2 * NT],
                        lhsT=kp[HD:P, mc * P : (mc + 1) * P],
                        rhs=qp[HD:P, nt_ * NT : (nt_ + 1) * NT],
                        start=True,
                        stop=True,
                    )
                nc.scalar.activation(
                    e_t[:, (mc % 4) * 2 * NT : ((mc % 4) + 1) * 2 * NT],
                    s_t,
                    EXP,
                    scale=SCALE,
                )
            return eA, eB

        def emit_pv_nt(p, nt_, e_h):
            """PV for BOTH heads of the pair over one n-half, the two chains
            interleaved per m-chunk in the two pv psum slots. Both heads pass
            mc 0-3 together, so the first e-half frees as early as possible
            for the next pair's S chain."""
            eA, eB = e_h
            hA, hB = 2 * p, 2 * p + 1
            psA = pv_psum.tile([HD + 1, NT], F32, tag="pv", name=f"pu{hA}_{nt_}")
            psB = pv_psum.tile([HD + 1, NT], F32, tag="pv", name=f"pu{hB}_{nt_}")
            for mc in range(NMT):
                e_t = eA if mc < 4 else eB
                for ps, j in ((psA, 0), (psB, 1)):
                    nc.tensor.matmul(
                        ps,
                        lhsT=vsb[mc][:, (2 * p + j) * (HD + 1) : (2 * p + j + 1) * (HD + 1)],
                        rhs=e_t[:, ((mc % 4) * 2 + j) * NT : ((mc % 4) * 2 + j + 1) * NT],
                        start=(mc == 0),
                        stop=(mc == NMT - 1),
                    )
            units = []
            for h, ps in ((hA, psA), (hB, psB)):
                u_t = u_pool.tile([HD + 1, NT], F32, tag="u", name=f"u{h}_{nt_}")
                nc.vector.tensor_copy(u_t, ps)
                units.append((h, u_t))
            emit_denorm(p, nt_, units)

        # ---------- output projection + bias ----------
        # dt halves paired on the stationary aot chunk, single [128, 1024]
        # psum per n-tile; alternate between the pj and s psum pools (the s
        # pool is free by the tail) to keep the tail double-buffered.
        wp_holder = []
        bias_holder = []

        def load_wp():
            bias_sb = one_pool.tile([P, C], F32, tag="bias", name="bias_sb")
            nc.scalar.dma_start(bias_sb, bias)
            bias_holder.append(bias_sb)
            wp_sb = w_pool.tile([P, NCH * C], BF16, tag="wst", name="wp_sb")
            wp_src = bass.AP(
                tensor=wpT.tensor,
                offset=0,
                ap=[[C, P], [P * C, NCH], [1, C]],
            )
            nc.scalar.dma_start(wp_sb, wp_src)
            wp_holder.append(wp_sb)

        def emit_outproj():
            bias_sb = bias_holder[0]
            wp_sb = wp_holder[0]
            for nt2 in range(NMT):
                pool = pj_psum if nt2 % 2 == 0 else s_psum
                ps = pool.tile(
                    [P, 2 * NT], F32, tag="pj" if nt2 % 2 == 0 else "s",
                    name=f"pso{nt2}",
                )
                o_sb = stage_pool.tile([P, 2 * NT], F32, tag="stage", name=f"o{nt2}")
                for ck in range(NCH):
                    for dt in range(NNT):
                        nc.tensor.matmul(
                            ps[:, dt * NT : (dt + 1) * NT],
                            lhsT=aot[ck][:, nt2 * P : (nt2 + 1) * P],
                            rhs=wp_sb[:, ck * C + dt * NT : ck * C + (dt + 1) * NT],
                            start=(ck == 0),
                            stop=(ck == NCH - 1),
                        )
                # finer drain granularity on the last tile shortens the
                # add->DMA tail after the final matmul
                nq = 4 if nt2 == NMT - 1 else 2
                qw = 2 * NT // nq
                for dq in range(nq):
                    nc.vector.tensor_add(
                        o_sb[:, dq * qw : (dq + 1) * qw],
                        ps[:, dq * qw : (dq + 1) * qw],
                        bias_sb[:, dq * qw : (dq + 1) * qw],
                    )
                    nc.sync.dma_start(
                        out[nt2 * P : (nt2 + 1) * P, dq * qw : (dq + 1) * qw],
                        o_sb[:, dq * qw : (dq + 1) * qw],
                    )

        # ---------- pipeline ----------
        # pair-0 qk-proj + pair-0 S/exp run while x/wqk1/wv stream in;
        # v-blocks follow, then PV(0) consumes them and the steady loop runs.
        qks = {}
        qp0, kp0 = emit_qkproj(0)
        e00 = emit_s_exp_nt(0, 0, qp0, kp0)
        e01 = emit_s_exp_nt(0, 1, qp0, kp0)
        qks[1] = emit_qkproj(1)
        load_wqk(2)
        for mt in range(NMT):
            emit_vblock(mt)
        emit_pv_nt(0, 0, e00)
        emit_pv_nt(0, 1, e01)
        qks[2] = emit_qkproj(2)
        for p in range(1, NPAIR):
            qp, kp = qks.pop(p)
            e0 = emit_s_exp_nt(p, 0, qp, kp)
            emit_pv_nt(p, 0, e0)
            e1 = emit_s_exp_nt(p, 1, qp, kp)
            emit_pv_nt(p, 1, e1)
            if p + 2 < NPAIR:
                load_wqk(p + 2)
                qks[p + 2] = emit_qkproj(p + 2)
            if p == 6:
                load_wp()
        emit_outproj()

    nc.compile()
    _optimize_pe_stream(nc)
    return nc


def make_in_maps(x, w_qkv, w_proj, b_proj):
    import ml_dtypes

    bf16 = ml_dtypes.bfloat16
    # packed per-pair qk weights: wqkP[p*128+r, ck*128 + j]       = Wq[p,j,ck,r]
    #                             wqkP[p*128+r, C + ck*128 + j]   = Wk[p,j,ck,r]
    Wq = w_qkv[:C].reshape(NPAIR, P, NCH, P)        # [p, j, ck, r]
    Wk = w_qkv[C : 2 * C].reshape(NPAIR, P, NCH, P)
    Aq = Wq.transpose(0, 3, 2, 1)                   # [p, r, ck, j]
    Ak = Wk.transpose(0, 3, 2, 1)
    wqkP = np.ascontiguousarray(
        np.concatenate([Aq.reshape(NPAIR, P, C), Ak.reshape(NPAIR, P, C)], axis=2)
        .reshape(NPAIR * P, 2 * C)
        .astype(bf16)
    )
    wvT = np.ascontiguousarray(w_qkv[2 * C :].T.astype(bf16))
    wpT = np.ascontiguousarray(w_proj.T.astype(bf16))
    bias_bc = np.ascontiguousarray(
        np.broadcast_to(b_proj, (P, C)).astype(np.float32)
    )
    in_maps = []
    for b in range(B):
        in_maps.append(
            {
                "xT": np.ascontiguousarray(x[b].T.astype(bf16)),
                "wqkP": wqkP,
                "wvT": wvT,
                "wpT": wpT,
                "bias_bc": bias_bc,
            }
        )
    return in_maps


_CACHED_NC = None


def kernel(x, w_qkv, w_proj, b_proj):
    global _CACHED_NC
    x = np.asarray(x, dtype=np.float32)
    w_qkv = np.asarray(w_qkv, dtype=np.float32)
    w_proj = np.asarray(w_proj, dtype=np.float32)
    b_proj = np.asarray(b_proj, dtype=np.float32)
    if _CACHED_NC is None:
        _CACHED_NC = build_module()
    nc = _CACHED_NC
    in_maps = make_in_maps(x, w_qkv, w_proj, b_proj)
    res = bass_utils.run_bass_kernel_spmd(nc, in_maps, core_ids=list(range(B)))
    return np.stack([res.results[b]["out"] for b in range(B)], axis=0)


if __name__ == "__main__":
    nc = build_module()
    ninst = sum(len(b.instructions) for b in nc.m.functions[0].blocks)
    print("module built ok;", ninst, "instructions")


# revision 42
# speedup vs baseline: 1.2134x; 1.0315x over previous
"""Multi-head attention (B=8, N=1024, C=1024, H=16) on 8 Trainium2 NeuronCores.

Sharding: pure data-parallel — one batch element per core, weights replicated,
no collectives.

v3 design (vs v2): startup DMA-issue fixes (per-pair packed wqk = 1 DMA/pair,
single 3D-AP DMAs for wv/wp, ones columns via memset instead of scatter DMA,
weight loads issued from the scalar-engine HWDGE queue so the sync queue only
carries x/wqk), pipeline reordered so pair-0 S+exp runs before the v-projection
(covers the wv DMA window), and the softmax-denominator broadcast done with
gpsimd partition_broadcast + DVE multiply instead of a DRAM bounce.

Per-core algorithm:
  v-proj:    v[m, dv] natural layout, interleaved [m, 16*(64+1)] with a ones
             column per head (PV then emits softmax denominators for free).
  qk-proj:   per pair p: qp[c(2 heads), n], kp[c, m] bf16 tiles in SBUF.
  attention: per (pair, nt half):
               S^T chunks for heads A,B into [128,1024] psum pairs,
               exp (ACT, 1024-wide, scale=1/8) -> eA/eB bf16 [128, 4096]
               PV: U_aug[65, nt] = v_aug.T @ expS accumulated over 8 m-chunks
             U -> SBUF (fp32, frees psum), den row 64 -> reciprocal ->
             partition_broadcast -> DVE multiply into attn_outT[c, n] bf16.
  out-proj:  out[n, d] = attn_outT.T @ wpT + bias, fp32 out.
"""

import sys

if "/opt/trn_rl_repo" not in sys.path:
    sys.path.insert(0, "/opt/trn_rl_repo")

from contextlib import ExitStack

import numpy as np

import concourse.bass as bass
import concourse.mybir as mybir
from concourse import bacc
import concourse.tile as tile
from concourse import bass_utils

B, N, C, H = 8, 1024, 1024, 16
HD = C // H          # 64
SCALE = HD ** -0.5   # 0.125
P = 128              # SBUF partitions
NT = 512             # moving-dim tile (fp32 PSUM bank limit)
NCH = C // P         # 8 contraction chunks over channels
NMT = N // P         # 8 token tiles of 128
NNT = N // NT        # 2 token tiles of 512
NPAIR = H // 2       # 8 head pairs
F32 = mybir.dt.float32
BF16 = mybir.dt.bfloat16
EXP = mybir.ActivationFunctionType.Exp


def _wait_key(w):
    return (w.sync_type, w.id, w.wait_mode, w.wait_value)


def _weights_sig(ldw):
    a = ldw.ins[0]
    return (a.memref, a.offset, tuple(tuple(x) for x in a.ap), str(a.dtype))


def _optimize_pe_stream(nc):
    """Post-compile peephole pass over the PE instruction stream.

    Operates on the PE-only subsequence (other engines' instructions are
    interleaved in the block list but the PE sequencer only sees its own
    stream; cross-engine ordering is carried entirely by semaphores).

    Rule 1 (dedupe): a wait-free LDWEIGHTS reloading the weights already in
      the array (and already consumed by a matmul) is deleted. Wait-carrying
      LDWs are kept: waits only function on LDWEIGHTS (the PE hw-decoder
      ignores waits on MATMUL), and the first load of a compiler
      [LDW, LDW, MM, MM] prefetch pair may run before the DMA-complete wait.
    Rule 2 (hoist): [LDW_A, MM_A@(0,0) K=64, LDW_B, MM_B@(64,0) K=64] ->
      [LDW_A, LDW_B, MM_A, MM_B] so the two matmuls execute concurrently on
      disjoint row groups; only when LDW_B's waits are implied by LDW_A's
      (same semaphore, same-or-lower threshold), so the earlier wait position
      cannot deadlock.
    """
    import concourse.mybir as mybir
    from collections import deque

    n_dedupe = n_hoist = 0
    for f in nc.m.functions:
        for blk in f.blocks:
            insts = blk.instructions
            pe = [a for a in insts if getattr(a, "engine", None) == mybir.EngineType.PE]
            out = []
            i = 0
            n = len(pe)
            cur_sig = None
            cur_consumed = False
            while i < n:
                a = pe[i]
                if isinstance(a, mybir.InstLdweights):
                    sig = _weights_sig(a)
                    if (
                        sig == cur_sig
                        and cur_consumed
                        and (
                            a.sync_info is None
                            or not (a.sync_info.on_wait or a.sync_info.on_update)
                        )
                        and i + 1 < n
                        and isinstance(pe[i + 1], mybir.InstMatmult)
                    ):
                        out.append(pe[i + 1])
                        i += 2
                        n_dedupe += 1
                        cur_consumed = True
                        continue
                    if (
                        i + 3 < n
                        and isinstance(pe[i + 1], mybir.InstMatmult)
                        and isinstance(pe[i + 2], mybir.InstLdweights)
                        and isinstance(pe[i + 3], mybir.InstMatmult)
                    ):
                        ldw1, mm1, ldw2, mm2 = pe[i : i + 4]
                        tp1 = mm1.tile_position
                        tp2 = mm2.tile_position
                        if (
                            tp1 is not None
                            and tp2 is not None
                            and tuple(tp1) == (0, 0)
                            and tuple(tp2) == (64, 0)
                            and ldw1.ins[0].ap[0][1] == 64
                            and ldw2.ins[0].ap[0][1] == 64
                        ):
                            w1 = [
                                _wait_key(w)
                                for w in (
                                    ldw1.sync_info.on_wait if ldw1.sync_info else []
                                )
                            ]
                            w2 = [
                                _wait_key(w)
                                for w in (
                                    ldw2.sync_info.on_wait if ldw2.sync_info else []
                                )
                            ]
                            implied = all(
                                any(
                                    k[0] == kk[0]
                                    and k[1] == kk[1]
                                    and k[2] == kk[2]
                                    and k[3] <= kk[3]
                                    for kk in w1
                                )
                                for k in w2
                            )
                            if implied and not (
                                ldw2.sync_info and ldw2.sync_info.on_update
                            ):
                                out.extend([ldw1, ldw2, mm1, mm2])
                                cur_sig = _weights_sig(ldw2)
                                cur_consumed = True
                                i += 4
                                n_hoist += 1
                                continue
                    cur_sig = sig
                    cur_consumed = False
                elif isinstance(a, mybir.InstMatmult):
                    cur_consumed = True
                else:
                    cur_sig = None  # unknown PE instruction: be conservative
                    cur_consumed = False
                out.append(a)
                i += 1
            # weave the transformed PE stream back into the block, keeping
            # non-PE instructions in place; trailing PE slots left over from
            # deletions are simply skipped.
            pe_q = deque(out)
            new_insts = []
            for a in insts:
                if getattr(a, "engine", None) == mybir.EngineType.PE:
                    if pe_q:
                        new_insts.append(pe_q.popleft())
                else:
                    new_insts.append(a)
            assert not pe_q, "transformed PE stream longer than original slots"
            blk.instructions = new_insts
    print(f"_optimize_pe_stream: {n_dedupe} LDW deduped, {n_hoist} LDW hoisted")


def build_module():
    nc = bacc.Bacc("TRN2", target_bir_lowering=False, debug=False, num_devices=B)

    xT = nc.dram_tensor("xT", [C, N], BF16, kind="ExternalInput").ap()
    # per-pair packed qk weights: row p*128+r, col ck*256 + (q j | k j)
    wqkP = nc.dram_tensor("wqkP", [NPAIR * P, NCH * 2 * P], BF16, kind="ExternalInput").ap()
    wvT = nc.dram_tensor("wvT", [C, C], BF16, kind="ExternalInput").ap()
    wpT = nc.dram_tensor("wpT", [C, C], BF16, kind="ExternalInput").ap()
    bias = nc.dram_tensor("bias_bc", [P, C], F32, kind="ExternalInput").ap()
    out = nc.dram_tensor("out", [N, C], F32, kind="ExternalOutput").ap()

    with tile.TileContext(nc) as tc, ExitStack() as ctx:
        dram = ctx.enter_context(tc.tile_pool(name="dram", bufs=1, space="DRAM"))
        rden_d = dram.tile([H * NNT, NT], F32, tag="rden_d", name="rden_d")

        xt_pool = ctx.enter_context(tc.tile_pool(name="xt", bufs=8))
        qk_pool = ctx.enter_context(tc.tile_pool(name="qk", bufs=7))
        v_pool = ctx.enter_context(tc.tile_pool(name="v", bufs=8))
        e_pool = ctx.enter_context(tc.tile_pool(name="e", bufs=6))
        u_pool = ctx.enter_context(tc.tile_pool(name="u", bufs=6))
        aot_pool = ctx.enter_context(tc.tile_pool(name="aot", bufs=1))
        w_pool = ctx.enter_context(tc.tile_pool(name="wst", bufs=2))
        wqk_pool = ctx.enter_context(tc.tile_pool(name="wqk", bufs=4))
        den_pool = ctx.enter_context(tc.tile_pool(name="den", bufs=4))
        rbc_pool = ctx.enter_context(tc.tile_pool(name="rbc", bufs=4))
        one_pool = ctx.enter_context(tc.tile_pool(name="one", bufs=1))
        stage_pool = ctx.enter_context(tc.tile_pool(name="stage", bufs=2))
        s_psum = ctx.enter_context(tc.tile_pool(name="s_ps", bufs=2, space="PSUM"))
        pv_psum = ctx.enter_context(tc.tile_pool(name="pv_ps", bufs=2, space="PSUM"))
        pj_psum = ctx.enter_context(tc.tile_pool(name="pj_ps", bufs=1, space="PSUM"))

        # ---------- qk weight load (packed per pair; 2 dmas each) ----------
        wqk_tiles = {}

        def load_wqk(p, eng=None):
            eng = eng or nc.sync
            w_t = wqk_pool.tile([P, NCH * 2 * P], BF16, tag="wqk", name=f"wqk{p}")
            # q half then k half so the first q matmul doesn't wait for k
            eng.dma_start(w_t[:, 0:C], wqkP[p * P : (p + 1) * P, 0:C])
            eng.dma_start(w_t[:, C : 2 * C], wqkP[p * P : (p + 1) * P, C : 2 * C])
            wqk_tiles[p] = w_t

        # ---------- input loads: x on sync queue, weights on scalar queue ----
        # few, large dma_starts: the DMA-semaphore ring only allows ~8-10
        # outstanding dma_starts, so a long run of small loads stalls the
        # issue queue for tens of us.
        # x per-chunk tiles, interleaved across both HWDGE queues so the 2MB
        # streams at full aggregate bandwidth; wqk0 q first (first MM dep),
        # wqk0 k and wqk1 woven in so each lands just before its matmuls.
        w0 = wqk_pool.tile([P, NCH * 2 * P], BF16, tag="wqk", name="wqk0")
        w1 = wqk_pool.tile([P, NCH * 2 * P], BF16, tag="wqk", name="wqk1")
        xts = [xt_pool.tile([P, N], BF16, tag="xt", name=f"xt{t}") for t in range(NCH)]
        nc.sync.dma_start(w0[:, 0:C], wqkP[0:P, 0:C])
        for t in range(0, NCH, 2):
            nc.sync.dma_start(xts[t], xT[t * P : (t + 1) * P, :])
        nc.sync.dma_start(w0[:, C : 2 * C], wqkP[0:P, C : 2 * C])
        nc.sync.dma_start(w1[:, 0:C], wqkP[P : 2 * P, 0:C])
        nc.sync.dma_start(w1[:, C : 2 * C], wqkP[P : 2 * P, C : 2 * C])
        for t in range(1, NCH, 2):
            nc.scalar.dma_start(xts[t], xT[t * P : (t + 1) * P, :])
        wqk_tiles[0] = w0
        wqk_tiles[1] = w1

        # wv as one [128, 8*1024] tile: block ck at cols [ck*1024, +1024)
        wv_sb = w_pool.tile([P, NCH * C], BF16, tag="wst", name="wv_sb")
        for h in range(2):
            wv_src = bass.AP(
                tensor=wvT.tensor,
                offset=h * 4 * P * C,
                ap=[[C, P], [P * C, 4], [1, C]],
            )
            nc.scalar.dma_start(wv_sb[:, h * 4 * C : (h + 1) * 4 * C], wv_src)

        vsb = []
        for mt in range(NMT):
            v_t = v_pool.tile([P, H * (HD + 1)], BF16, tag="v", name=f"v{mt}")
            # contiguous full-tile fill; the v-proj copy then overwrites the
            # value columns, leaving 1.0 in each head's 65th (ones) column
            nc.gpsimd.memset(v_t, 1.0)
            vsb.append(v_t)
        aot = [
            aot_pool.tile([P, N], BF16, tag=f"aot{t}", name=f"aot{t}")
            for t in range(NCH)
        ]

        # ---------- v projection (natural layout + ones cols) ----------
        def emit_vblock(mt):
                ps = pj_psum.tile([P, 2 * NT], F32, tag="pj", name=f"psv{mt}")
                for ck in range(NCH):
                    for dvt in range(NNT):
                        nc.tensor.matmul(
                            ps[:, dvt * NT : (dvt + 1) * NT],
                            lhsT=xts[ck][:, mt * P : (mt + 1) * P],
                            rhs=wv_sb[:, ck * C + dvt * NT : ck * C + (dvt + 1) * NT],
                            start=(ck == 0),
                            stop=(ck == NCH - 1),
                        )
                for dvt in range(NNT):
                    nc.vector.tensor_copy(
                        vsb[mt].rearrange("p (h w) -> p h w", w=HD + 1)[
                            :, dvt * NCH : (dvt + 1) * NCH, 0:HD
                        ],
                        ps[:, dvt * NT : (dvt + 1) * NT].rearrange(
                            "p (h w) -> p h w", w=HD
                        ),
                    )

        # ---------- qk projection for one head pair, SBUF-resident ----------
        def emit_qkproj(p):
            qp = qk_pool.tile([P, N], BF16, tag="qk", name=f"qp{p}")
            kp = qk_pool.tile([P, N], BF16, tag="qk", name=f"kp{p}")
            w_t = wqk_tiles.pop(p)
            for which, dstt in ((0, qp), (1, kp)):
                ps = pj_psum.tile([P, 2 * NT], F32, tag="pj", name=f"psqk{which}_{p}")
                for ck in range(NCH):
                    for nt_ in range(NNT):
                        nc.tensor.matmul(
                            ps[:, nt_ * NT : (nt_ + 1) * NT],
                            lhsT=w_t[:, which * C + ck * P : which * C + (ck + 1) * P],
                            rhs=xts[ck][:, nt_ * NT : (nt_ + 1) * NT],
                            start=(ck == 0),
                            stop=(ck == NCH - 1),
                        )
                for nt_ in range(NNT):
                    nc.vector.tensor_copy(
                        dstt[:, nt_ * NT : (nt_ + 1) * NT],
                        ps[:, nt_ * NT : (nt_ + 1) * NT],
                    )
            return qp, kp

        # ---------- attention ----------
        pair_units = {}

        def emit_denorm(p, nt_, punits):
            """den rows -> reciprocal -> DRAM bounce -> partition-broadcast
            read -> DVE multiply into the attn-out tiles (bf16 cast on write).
            Runs per (pair, nt-half) so the out-proj's first n-half unblocks
            as soon as the last pair's nt=0 units are normalized."""
            g = p * 4 + nt_ * 2
            den_g = den_pool.tile([2, NT], F32, tag="den", name=f"den{p}_{nt_}")
            for i, (h, u_t) in enumerate(punits):
                nc.sync.dma_start(den_g[i : i + 1, :], u_t[HD : HD + 1, :])
            rden = den_pool.tile([2, NT], F32, tag="rden", name=f"rden{p}_{nt_}")
            nc.vector.reciprocal_approx_fast(out=rden, in_=den_g)
            nc.sync.dma_start(rden_d[g : g + 2, :], rden)
            for i, (h, u_t) in enumerate(punits):
                rbc = rbc_pool.tile([HD, NT], F32, tag="rbc", name=f"rbc{h}_{nt_}")
                src_ = rden_d[g + i : g + i + 1, :]
                bsrc = bass.AP(
                    tensor=src_.tensor,
                    offset=src_.offset,
                    ap=[[0, HD], list(src_.ap[-1])],
                )
                nc.sync.dma_start(out=rbc, in_=bsrc)
                ct, prow = h // 2, (h % 2) * HD
                nc.vector.tensor_mul(
                    aot[ct][prow : prow + HD, nt_ * NT : (nt_ + 1) * NT],
                    u_t[0:HD, :],
                    rbc,
                )

        def emit_s_exp_nt(p, nt_, qp, kp):
            """S^T + exp for one (pair, n-half). Heads A and B share one
            [128, 1024] psum tile per m-chunk (A in the low bank, B in the
            high bank) so both matmuls become ready together; the post-compile
            pass hoists B's LDWEIGHTS above A's matmul, making the two K=64
            matmuls (array rows 0-63 / 64-127) run concurrently.
            Returns two e tiles [128, 4096] (mc 0-3 and mc 4-7), each laid
            out as [A_mc|B_mc|...]; the split lets PV release the first half
            mid-chain so the next pair's S can reuse the slots earlier."""
            eA = e_pool.tile([P, NMT * NT], BF16, tag="e", name=f"e{p}_{nt_}a")
            eB = e_pool.tile([P, NMT * NT], BF16, tag="e", name=f"e{p}_{nt_}b")
            for mc in range(NMT):
                e_t = eA if mc < 4 else eB
                s_t = s_psum.tile([P, 2 * NT], F32, tag="s", name=f"s{p}_{nt_}_{mc}")
                # high priority: the S pair feeds ACT (the attention-phase
                # pacer) and must pop back-to-back so the post-compile hoist
                # can make the two K=64 row-tiles run concurrently.
                with tc.high_priority():
                    nc.tensor.matmul(
                        s_t[:, 0:NT],
                        lhsT=kp[0:HD, mc * P : (mc + 1) * P],
                        rhs=qp[0:HD, nt_ * NT : (nt_ + 1) * NT],
                        start=True,
                        stop=True,
                    )
                    nc.tensor.matmul(
                        s_t[:, NT : 2 * NT],
                        lhsT=kp[HD:P, mc * P : (mc + 1) * P],
                        rhs=qp[HD:P, nt_ * NT : (nt_ + 1) * NT],
                        start=True,
                        stop=True,
                    )
                nc.scalar.activation(
                    e_t[:, (mc % 4) * 2 * NT : ((mc % 4) + 1) * 2 * NT],
                    s_t,
                    EXP,
                    scale=SCALE,
                )
            return eA, eB

        def emit_pv_nt(p, nt_, e_h):
            """PV for BOTH heads of the pair over one n-half, the two chains
            interleaved per m-chunk in the two pv psum slots. Both heads pass
            mc 0-3 together, so the first e-half frees as early as possible
            for the next pair's S chain."""
            eA, eB = e_h
            hA, hB = 2 * p, 2 * p + 1
            psA = pv_psum.tile([HD + 1, NT], F32, tag="pv", name=f"pu{hA}_{nt_}")
            psB = pv_psum.tile([HD + 1, NT], F32, tag="pv", name=f"pu{hB}_{nt_}")
            for mc in range(NMT):
                e_t = eA if mc < 4 else eB
                for ps, j in ((psA, 0), (psB, 1)):
                    nc.tensor.matmul(
                        ps,
                        lhsT=vsb[mc][:, (2 * p + j) * (HD + 1) : (2 * p + j + 1) * (HD + 1)],
                        rhs=e_t[:, ((mc % 4) * 2 + j) * NT : ((mc % 4) * 2 + j + 1) * NT],
                        start=(mc == 0),
                        stop=(mc == NMT - 1),
                    )
            units = []
            for h, ps in ((hA, psA), (hB, psB)):
                u_t = u_pool.tile([HD + 1, NT], F32, tag="u", name=f"u{h}_{nt_}")
                nc.vector.tensor_copy(u_t, ps)
                units.append((h, u_t))
            emit_denorm(p, nt_, units)

        # ---------- output projection + bias ----------
        # dt halves paired on the stationary aot chunk, single [128, 1024]
        # psum per n-tile; alternate between the pj and s psum pools (the s
        # pool is free by the tail) to keep the tail double-buffered.
        wp_holder = []
        bias_holder = []

        def load_wp():
            bias_sb = one_pool.tile([P, C], F32, tag="bias", name="bias_sb")
            nc.scalar.dma_start(bias_sb, bias)
            bias_holder.append(bias_sb)
            wp_sb = w_pool.tile([P, NCH * C], BF16, tag="wst", name="wp_sb")
            wp_src = bass.AP(
                tensor=wpT.tensor,
                offset=0,
                ap=[[C, P], [P * C, NCH], [1, C]],
            )
            nc.scalar.dma_start(wp_sb, wp_src)
            wp_holder.append(wp_sb)

        def emit_outproj():
            bias_sb = bias_holder[0]
            wp_sb = wp_holder[0]
            for nt2 in range(NMT):
                pool = pj_psum if nt2 % 2 == 0 else s_psum
                ps = pool.tile(
                    [P, 2 * NT], F32, tag="pj" if nt2 % 2 == 0 else "s",
                    name=f"pso{nt2}",
                )
                o_sb = stage_pool.tile([P, 2 * NT], F32, tag="stage", name=f"o{nt2}")
                for ck in range(NCH):
                    for dt in range(NNT):
                        nc.tensor.matmul(
                            ps[:, dt * NT : (dt + 1) * NT],
                            lhsT=aot[ck][:, nt2 * P : (nt2 + 1) * P],
                            rhs=wp_sb[:, ck * C + dt * NT : ck * C + (dt + 1) * NT],
                            start=(ck == 0),
                            stop=(ck == NCH - 1),
                        )
                nq = 2
                qw = 2 * NT // nq
                for dq in range(nq):
                    nc.vector.tensor_add(
                        o_sb[:, dq * qw : (dq + 1) * qw],
                        ps[:, dq * qw : (dq + 1) * qw],
                        bias_sb[:, dq * qw : (dq + 1) * qw],
                    )
                    nc.sync.dma_start(
                        out[nt2 * P : (nt2 + 1) * P, dq * qw : (dq + 1) * qw],
                        o_sb[:, dq * qw : (dq + 1) * qw],
                    )

        # ---------- pipeline ----------
        # pair-0 qk-proj + pair-0 S/exp run while x/wqk1/wv stream in;
        # v-blocks follow, then PV(0) consumes them and the steady loop runs.
        qks = {}
        qp0, kp0 = emit_qkproj(0)
        e00 = emit_s_exp_nt(0, 0, qp0, kp0)
        e01 = emit_s_exp_nt(0, 1, qp0, kp0)
        qks[1] = emit_qkproj(1)
        load_wqk(2)
        for mt in range(NMT):
            emit_vblock(mt)
        emit_pv_nt(0, 0, e00)
        emit_pv_nt(0, 1, e01)
        qks[2] = emit_qkproj(2)
        for p in range(1, NPAIR):
            qp, kp = qks.pop(p)
            e0 = emit_s_exp_nt(p, 0, qp, kp)
            emit_pv_nt(p, 0, e0)
            e1 = emit_s_exp_nt(p, 1, qp, kp)
            emit_pv_nt(p, 1, e1)
            if p + 2 < NPAIR:
                load_wqk(p + 2)
                qks[p + 2] = emit_qkproj(p + 2)
            if p == 6:
                load_wp()
        emit_outproj()

    nc.compile()
    _optimize_pe_stream(nc)
    return nc


def make_in_maps(x, w_qkv, w_proj, b_proj):
    import ml_dtypes

    bf16 = ml_dtypes.bfloat16
    # packed per-pair qk weights: wqkP[p*128+r, ck*128 + j]       = Wq[p,j,ck,r]
    #                             wqkP[p*128+r, C + ck*128 + j]   = Wk[p,j,ck,r]
    Wq = w_qkv[:C].reshape(NPAIR, P, NCH, P)        # [p, j, ck, r]
    Wk = w_qkv[C : 2 * C].reshape(NPAIR, P, NCH, P)
    Aq = Wq.transpose(0, 3, 2, 1)                   # [p, r, ck, j]
    Ak = Wk.transpose(0, 3, 2, 1)
    wqkP = np.ascontiguousarray(
        np.concatenate([Aq.reshape(NPAIR, P, C), Ak.reshape(NPAIR, P, C)], axis=2)
        .reshape(NPAIR * P, 2 * C)
        .astype(bf16)
    )
    wvT = np.ascontiguousarray(w_qkv[2 * C :].T.astype(bf16))
    wpT = np.ascontiguousarray(w_proj.T.astype(bf16))
    bias_bc = np.ascontiguousarray(
        np.broadcast_to(b_proj, (P, C)).astype(np.float32)
    )
    in_maps = []
    for b in range(B):
        in_maps.append(
            {
                "xT": np.ascontiguousarray(x[b].T.astype(bf16)),
                "wqkP": wqkP,
                "wvT": wvT,
                "wpT": wpT,
                "bias_bc": bias_bc,
            }
        )
    return in_maps


_CACHED_NC = None


def kernel(x, w_qkv, w_proj, b_proj):
    global _CACHED_NC
    x = np.asarray(x, dtype=np.float32)
    w_qkv = np.asarray(w_qkv, dtype=np.float32)
    w_proj = np.asarray(w_proj, dtype=np.float32)
    b_proj = np.asarray(b_proj, dtype=np.float32)
    if _CACHED_NC is None:
        _CACHED_NC = build_module()
    nc = _CACHED_NC
    in_maps = make_in_maps(x, w_qkv, w_proj, b_proj)
    res = bass_utils.run_bass_kernel_spmd(nc, in_maps, core_ids=list(range(B)))
    return np.stack([res.results[b]["out"] for b in range(B)], axis=0)


if __name__ == "__main__":
    nc = build_module()
    ninst = sum(len(b.instructions) for b in nc.m.functions[0].blocks)
    print("module built ok;", ninst, "instructions")


# revision 45
# speedup vs baseline: 1.2404x; 1.0223x over previous
"""Multi-head attention (B=8, N=1024, C=1024, H=16) on 8 Trainium2 NeuronCores.

Sharding: pure data-parallel — one batch element per core, weights replicated,
no collectives.

v3 design (vs v2): startup DMA-issue fixes (per-pair packed wqk = 1 DMA/pair,
single 3D-AP DMAs for wv/wp, ones columns via memset instead of scatter DMA,
weight loads issued from the scalar-engine HWDGE queue so the sync queue only
carries x/wqk), pipeline reordered so pair-0 S+exp runs before the v-projection
(covers the wv DMA window), and the softmax-denominator broadcast done with
gpsimd partition_broadcast + DVE multiply instead of a DRAM bounce.

Per-core algorithm:
  v-proj:    v[m, dv] natural layout, interleaved [m, 16*(64+1)] with a ones
             column per head (PV then emits softmax denominators for free).
  qk-proj:   per pair p: qp[c(2 heads), n], kp[c, m] bf16 tiles in SBUF.
  attention: per (pair, nt half):
               S^T chunks for heads A,B into [128,1024] psum pairs,
               exp (ACT, 1024-wide, scale=1/8) -> eA/eB bf16 [128, 4096]
               PV: U_aug[65, nt] = v_aug.T @ expS accumulated over 8 m-chunks
             U -> SBUF (fp32, frees psum), den row 64 -> reciprocal ->
             partition_broadcast -> DVE multiply into attn_outT[c, n] bf16.
  out-proj:  out[n, d] = attn_outT.T @ wpT + bias, fp32 out.
"""

import sys

if "/opt/trn_rl_repo" not in sys.path:
    sys.path.insert(0, "/opt/trn_rl_repo")

from contextlib import ExitStack

import numpy as np

import concourse.bass as bass
import concourse.mybir as mybir
from concourse import bacc
import concourse.tile as tile
from concourse import bass_utils

B, N, C, H = 8, 1024, 1024, 16
HD = C // H          # 64
SCALE = HD ** -0.5   # 0.125
P = 128              # SBUF partitions
NT = 512             # moving-dim tile (fp32 PSUM bank limit)
NCH = C // P         # 8 contraction chunks over channels
NMT = N // P         # 8 token tiles of 128
NNT = N // NT        # 2 token tiles of 512
NPAIR = H // 2       # 8 head pairs
F32 = mybir.dt.float32
BF16 = mybir.dt.bfloat16
EXP = mybir.ActivationFunctionType.Exp


def _wait_key(w):
    return (w.sync_type, w.id, w.wait_mode, w.wait_value)


def _weights_sig(ldw):
    a = ldw.ins[0]
    return (a.memref, a.offset, tuple(tuple(x) for x in a.ap), str(a.dtype))


def _optimize_pe_stream(nc):
    """Post-compile peephole pass over the PE instruction stream.

    Operates on the PE-only subsequence (other engines' instructions are
    interleaved in the block list but the PE sequencer only sees its own
    stream; cross-engine ordering is carried entirely by semaphores).

    Rule 1 (dedupe): a wait-free LDWEIGHTS reloading the weights already in
      the array (and already consumed by a matmul) is deleted. Wait-carrying
      LDWs are kept: waits only function on LDWEIGHTS (the PE hw-decoder
      ignores waits on MATMUL), and the first load of a compiler
      [LDW, LDW, MM, MM] prefetch pair may run before the DMA-complete wait.
    Rule 2 (hoist): [LDW_A, MM_A@(0,0) K=64, LDW_B, MM_B@(64,0) K=64] ->
      [LDW_A, LDW_B, MM_A, MM_B] so the two matmuls execute concurrently on
      disjoint row groups; only when LDW_B's waits are implied by LDW_A's
      (same semaphore, same-or-lower threshold), so the earlier wait position
      cannot deadlock.
    """
    import concourse.mybir as mybir
    from collections import deque

    n_dedupe = n_hoist = 0
    for f in nc.m.functions:
        for blk in f.blocks:
            insts = blk.instructions
            pe = [a for a in insts if getattr(a, "engine", None) == mybir.EngineType.PE]
            out = []
            i = 0
            n = len(pe)
            cur_sig = None
            cur_consumed = False
            while i < n:
                a = pe[i]
                if isinstance(a, mybir.InstLdweights):
                    sig = _weights_sig(a)
                    if (
                        sig == cur_sig
                        and cur_consumed
                        and (
                            a.sync_info is None
                            or not (a.sync_info.on_wait or a.sync_info.on_update)
                        )
                        and i + 1 < n
                        and isinstance(pe[i + 1], mybir.InstMatmult)
                    ):
                        out.append(pe[i + 1])
                        i += 2
                        n_dedupe += 1
                        cur_consumed = True
                        continue
                    if (
                        i + 3 < n
                        and isinstance(pe[i + 1], mybir.InstMatmult)
                        and isinstance(pe[i + 2], mybir.InstLdweights)
                        and isinstance(pe[i + 3], mybir.InstMatmult)
                    ):
                        ldw1, mm1, ldw2, mm2 = pe[i : i + 4]
                        tp1 = mm1.tile_position
                        tp2 = mm2.tile_position
                        if (
                            tp1 is not None
                            and tp2 is not None
                            and tuple(tp1) == (0, 0)
                            and tuple(tp2) == (64, 0)
                            and ldw1.ins[0].ap[0][1] == 64
                            and ldw2.ins[0].ap[0][1] == 64
                        ):
                            w1 = [
                                _wait_key(w)
                                for w in (
                                    ldw1.sync_info.on_wait if ldw1.sync_info else []
                                )
                            ]
                            w2 = [
                                _wait_key(w)
                                for w in (
                                    ldw2.sync_info.on_wait if ldw2.sync_info else []
                                )
                            ]
                            implied = all(
                                any(
                                    k[0] == kk[0]
                                    and k[1] == kk[1]
                                    and k[2] == kk[2]
                                    and k[3] <= kk[3]
                                    for kk in w1
                                )
                                for k in w2
                            )
                            if implied and not (
                                ldw2.sync_info and ldw2.sync_info.on_update
                            ):
                                out.extend([ldw1, ldw2, mm1, mm2])
                                cur_sig = _weights_sig(ldw2)
                                cur_consumed = True
                                i += 4
                                n_hoist += 1
                                continue
                    cur_sig = sig
                    cur_consumed = False
                elif isinstance(a, mybir.InstMatmult):
                    cur_consumed = True
                else:
                    cur_sig = None  # unknown PE instruction: be conservative
                    cur_consumed = False
                out.append(a)
                i += 1
            # weave the transformed PE stream back into the block, keeping
            # non-PE instructions in place; trailing PE slots left over from
            # deletions are simply skipped.
            pe_q = deque(out)
            new_insts = []
            for a in insts:
                if getattr(a, "engine", None) == mybir.EngineType.PE:
                    if pe_q:
                        new_insts.append(pe_q.popleft())
                else:
                    new_insts.append(a)
            assert not pe_q, "transformed PE stream longer than original slots"
            blk.instructions = new_insts
    print(f"_optimize_pe_stream: {n_dedupe} LDW deduped, {n_hoist} LDW hoisted")


def build_module():
    nc = bacc.Bacc("TRN2", target_bir_lowering=False, debug=False, num_devices=B)

    xT = nc.dram_tensor("xT", [C, N], BF16, kind="ExternalInput").ap()
    # per-pair packed qk weights: row p*128+r, col ck*256 + (q j | k j)
    wqkP = nc.dram_tensor("wqkP", [NPAIR * P, NCH * 2 * P], BF16, kind="ExternalInput").ap()
    wvT = nc.dram_tensor("wvT", [C, C], BF16, kind="ExternalInput").ap()
    wpT = nc.dram_tensor("wpT", [C, C], BF16, kind="ExternalInput").ap()
    bias = nc.dram_tensor("bias_bc", [P, C], F32, kind="ExternalInput").ap()
    out = nc.dram_tensor("out", [N, C], F32, kind="ExternalOutput").ap()

    with tile.TileContext(nc) as tc, ExitStack() as ctx:
        dram = ctx.enter_context(tc.tile_pool(name="dram", bufs=1, space="DRAM"))
        rden_d = dram.tile([H * NNT, NT], F32, tag="rden_d", name="rden_d")

        xt_pool = ctx.enter_context(tc.tile_pool(name="xt", bufs=8))
        qk_pool = ctx.enter_context(tc.tile_pool(name="qk", bufs=7))
        v_pool = ctx.enter_context(tc.tile_pool(name="v", bufs=8))
        e_pool = ctx.enter_context(tc.tile_pool(name="e", bufs=6))
        u_pool = ctx.enter_context(tc.tile_pool(name="u", bufs=6))
        aot_pool = ctx.enter_context(tc.tile_pool(name="aot", bufs=1))
        w_pool = ctx.enter_context(tc.tile_pool(name="wst", bufs=2))
        wqk_pool = ctx.enter_context(tc.tile_pool(name="wqk", bufs=4))
        rbc_pool = ctx.enter_context(tc.tile_pool(name="rbc", bufs=6))
        one_pool = ctx.enter_context(tc.tile_pool(name="one", bufs=1))
        stage_pool = ctx.enter_context(tc.tile_pool(name="stage", bufs=2))
        s_psum = ctx.enter_context(tc.tile_pool(name="s_ps", bufs=2, space="PSUM"))
        pv_psum = ctx.enter_context(tc.tile_pool(name="pv_ps", bufs=2, space="PSUM"))
        pj_psum = ctx.enter_context(tc.tile_pool(name="pj_ps", bufs=1, space="PSUM"))

        # ---------- qk weight load (packed per pair; 2 dmas each) ----------
        wqk_tiles = {}

        def load_wqk(p, eng=None):
            eng = eng or nc.sync
            w_t = wqk_pool.tile([P, NCH * 2 * P], BF16, tag="wqk", name=f"wqk{p}")
            # q half then k half so the first q matmul doesn't wait for k
            eng.dma_start(w_t[:, 0:C], wqkP[p * P : (p + 1) * P, 0:C])
            eng.dma_start(w_t[:, C : 2 * C], wqkP[p * P : (p + 1) * P, C : 2 * C])
            wqk_tiles[p] = w_t

        # ---------- input loads: x on sync queue, weights on scalar queue ----
        # few, large dma_starts: the DMA-semaphore ring only allows ~8-10
        # outstanding dma_starts, so a long run of small loads stalls the
        # issue queue for tens of us.
        # x per-chunk tiles, interleaved across both HWDGE queues so the 2MB
        # streams at full aggregate bandwidth; wqk0 q first (first MM dep),
        # wqk0 k and wqk1 woven in so each lands just before its matmuls.
        w0 = wqk_pool.tile([P, NCH * 2 * P], BF16, tag="wqk", name="wqk0")
        w1 = wqk_pool.tile([P, NCH * 2 * P], BF16, tag="wqk", name="wqk1")
        xts = [xt_pool.tile([P, N], BF16, tag="xt", name=f"xt{t}") for t in range(NCH)]
        nc.sync.dma_start(w0[:, 0:C], wqkP[0:P, 0:C])
        for t in range(0, NCH, 2):
            nc.sync.dma_start(xts[t], xT[t * P : (t + 1) * P, :])
        nc.sync.dma_start(w0[:, C : 2 * C], wqkP[0:P, C : 2 * C])
        nc.sync.dma_start(w1[:, 0:C], wqkP[P : 2 * P, 0:C])
        nc.sync.dma_start(w1[:, C : 2 * C], wqkP[P : 2 * P, C : 2 * C])
        for t in range(1, NCH, 2):
            nc.scalar.dma_start(xts[t], xT[t * P : (t + 1) * P, :])
        wqk_tiles[0] = w0
        wqk_tiles[1] = w1

        # wv as one [128, 8*1024] tile: block ck at cols [ck*1024, +1024)
        wv_sb = w_pool.tile([P, NCH * C], BF16, tag="wst", name="wv_sb")
        for h in range(2):
            wv_src = bass.AP(
                tensor=wvT.tensor,
                offset=h * 4 * P * C,
                ap=[[C, P], [P * C, 4], [1, C]],
            )
            nc.scalar.dma_start(wv_sb[:, h * 4 * C : (h + 1) * 4 * C], wv_src)

        vsb = []
        for mt in range(NMT):
            v_t = v_pool.tile([P, H * (HD + 1)], BF16, tag="v", name=f"v{mt}")
            # contiguous full-tile fill; the v-proj copy then overwrites the
            # value columns, leaving 1.0 in each head's 65th (ones) column
            nc.gpsimd.memset(v_t, 1.0)
            vsb.append(v_t)
        aot = [
            aot_pool.tile([P, N], BF16, tag=f"aot{t}", name=f"aot{t}")
            for t in range(NCH)
        ]

        # ---------- v projection (natural layout + ones cols) ----------
        def emit_vblock(mt):
                ps = pj_psum.tile([P, 2 * NT], F32, tag="pj", name=f"psv{mt}")
                for ck in range(NCH):
                    for dvt in range(NNT):
                        nc.tensor.matmul(
                            ps[:, dvt * NT : (dvt + 1) * NT],
                            lhsT=xts[ck][:, mt * P : (mt + 1) * P],
                            rhs=wv_sb[:, ck * C + dvt * NT : ck * C + (dvt + 1) * NT],
                            start=(ck == 0),
                            stop=(ck == NCH - 1),
                        )
                for dvt in range(NNT):
                    nc.vector.tensor_copy(
                        vsb[mt].rearrange("p (h w) -> p h w", w=HD + 1)[
                            :, dvt * NCH : (dvt + 1) * NCH, 0:HD
                        ],
                        ps[:, dvt * NT : (dvt + 1) * NT].rearrange(
                            "p (h w) -> p h w", w=HD
                        ),
                    )

        # ---------- qk projection for one head pair, SBUF-resident ----------
        def emit_qkproj(p):
            qp = qk_pool.tile([P, N], BF16, tag="qk", name=f"qp{p}")
            kp = qk_pool.tile([P, N], BF16, tag="qk", name=f"kp{p}")
            w_t = wqk_tiles.pop(p)
            for which, dstt in ((0, qp), (1, kp)):
                ps = pj_psum.tile([P, 2 * NT], F32, tag="pj", name=f"psqk{which}_{p}")
                for ck in range(NCH):
                    for nt_ in range(NNT):
                        nc.tensor.matmul(
                            ps[:, nt_ * NT : (nt_ + 1) * NT],
                            lhsT=w_t[:, which * C + ck * P : which * C + (ck + 1) * P],
                            rhs=xts[ck][:, nt_ * NT : (nt_ + 1) * NT],
                            start=(ck == 0),
                            stop=(ck == NCH - 1),
                        )
                for nt_ in range(NNT):
                    nc.vector.tensor_copy(
                        dstt[:, nt_ * NT : (nt_ + 1) * NT],
                        ps[:, nt_ * NT : (nt_ + 1) * NT],
                    )
            return qp, kp

        # ---------- attention ----------
        pair_units = {}

        def emit_denorm(p, nt_, punits):
            """den rows -> DRAM bounce -> partition-broadcast read ->
            reciprocal -> DVE multiply into the attn-out tiles (bf16 cast on
            write). The reciprocal runs AFTER the DMA round-trip so the
            exposed chain is one SBUF->DRAM->SBUF hop plus two DVE ops.
            Runs per (pair, nt-half) so the out-proj's first n-half unblocks
            as soon as the last pair's nt=0 units are normalized."""
            g = p * 4 + nt_ * 2
            for i, (h, u_t) in enumerate(punits):
                nc.sync.dma_start(rden_d[g + i : g + i + 1, :], u_t[HD : HD + 1, :])
            for i, (h, u_t) in enumerate(punits):
                rbc = rbc_pool.tile([HD, NT], F32, tag="rbc", name=f"rbc{h}_{nt_}")
                src_ = rden_d[g + i : g + i + 1, :]
                bsrc = bass.AP(
                    tensor=src_.tensor,
                    offset=src_.offset,
                    ap=[[0, HD], list(src_.ap[-1])],
                )
                nc.sync.dma_start(out=rbc, in_=bsrc)
                rr = rbc_pool.tile([HD, NT], F32, tag="rbc", name=f"rr{h}_{nt_}")
                nc.vector.reciprocal_approx_fast(out=rr, in_=rbc)
                ct, prow = h // 2, (h % 2) * HD
                nc.vector.tensor_mul(
                    aot[ct][prow : prow + HD, nt_ * NT : (nt_ + 1) * NT],
                    u_t[0:HD, :],
                    rr,
                )

        def emit_s_exp_nt(p, nt_, qp, kp):
            """S^T + exp for one (pair, n-half). Heads A and B share one
            [128, 1024] psum tile per m-chunk (A in the low bank, B in the
            high bank) so both matmuls become ready together; the post-compile
            pass hoists B's LDWEIGHTS above A's matmul, making the two K=64
            matmuls (array rows 0-63 / 64-127) run concurrently.
            Returns two e tiles [128, 4096] (mc 0-3 and mc 4-7), each laid
            out as [A_mc|B_mc|...]; the split lets PV release the first half
            mid-chain so the next pair's S can reuse the slots earlier."""
            eA = e_pool.tile([P, NMT * NT], BF16, tag="e", name=f"e{p}_{nt_}a")
            eB = e_pool.tile([P, NMT * NT], BF16, tag="e", name=f"e{p}_{nt_}b")
            for mc in range(NMT):
                e_t = eA if mc < 4 else eB
                s_t = s_psum.tile([P, 2 * NT], F32, tag="s", name=f"s{p}_{nt_}_{mc}")
                # high priority: the S pair feeds ACT (the attention-phase
                # pacer) and must pop back-to-back so the post-compile hoist
                # can make the two K=64 row-tiles run concurrently.
                with tc.high_priority():
                    nc.tensor.matmul(
                        s_t[:, 0:NT],
                        lhsT=kp[0:HD, mc * P : (mc + 1) * P],
                        rhs=qp[0:HD, nt_ * NT : (nt_ + 1) * NT],
                        start=True,
                        stop=True,
                    )
                    nc.tensor.matmul(
                        s_t[:, NT : 2 * NT],
                        lhsT=kp[HD:P, mc * P : (mc + 1) * P],
                        rhs=qp[HD:P, nt_ * NT : (nt_ + 1) * NT],
                        start=True,
                        stop=True,
                    )
                nc.scalar.activation(
                    e_t[:, (mc % 4) * 2 * NT : ((mc % 4) + 1) * 2 * NT],
                    s_t,
                    EXP,
                    scale=SCALE,
                )
            return eA, eB

        def emit_pv_nt(p, nt_, e_h):
            """PV for BOTH heads of the pair over one n-half, the two chains
            interleaved per m-chunk in the two pv psum slots. Both heads pass
            mc 0-3 together, so the first e-half frees as early as possible
            for the next pair's S chain."""
            eA, eB = e_h
            hA, hB = 2 * p, 2 * p + 1
            psA = pv_psum.tile([HD + 1, NT], F32, tag="pv", name=f"pu{hA}_{nt_}")
            psB = pv_psum.tile([HD + 1, NT], F32, tag="pv", name=f"pu{hB}_{nt_}")
            for mc in range(NMT):
                e_t = eA if mc < 4 else eB
                for ps, j in ((psA, 0), (psB, 1)):
                    nc.tensor.matmul(
                        ps,
                        lhsT=vsb[mc][:, (2 * p + j) * (HD + 1) : (2 * p + j + 1) * (HD + 1)],
                        rhs=e_t[:, ((mc % 4) * 2 + j) * NT : ((mc % 4) * 2 + j + 1) * NT],
                        start=(mc == 0),
                        stop=(mc == NMT - 1),
                    )
            units = []
            for h, ps in ((hA, psA), (hB, psB)):
                u_t = u_pool.tile([HD + 1, NT], F32, tag="u", name=f"u{h}_{nt_}")
                nc.vector.tensor_copy(u_t, ps)
                units.append((h, u_t))
            emit_denorm(p, nt_, units)

        # ---------- output projection + bias ----------
        # dt halves paired on the stationary aot chunk, single [128, 1024]
        # psum per n-tile; alternate between the pj and s psum pools (the s
        # pool is free by the tail) to keep the tail double-buffered.
        wp_holder = []
        bias_holder = []

        def load_wp():
            bias_sb = one_pool.tile([P, C], F32, tag="bias", name="bias_sb")
            nc.scalar.dma_start(bias_sb, bias)
            bias_holder.append(bias_sb)
            wp_sb = w_pool.tile([P, NCH * C], BF16, tag="wst", name="wp_sb")
            wp_src = bass.AP(
                tensor=wpT.tensor,
                offset=0,
                ap=[[C, P], [P * C, NCH], [1, C]],
            )
            nc.scalar.dma_start(wp_sb, wp_src)
            wp_holder.append(wp_sb)

        def emit_outproj():
            bias_sb = bias_holder[0]
            wp_sb = wp_holder[0]
            for nt2 in range(NMT):
                pool = pj_psum if nt2 % 2 == 0 else s_psum
                ps = pool.tile(
                    [P, 2 * NT], F32, tag="pj" if nt2 % 2 == 0 else "s",
                    name=f"pso{nt2}",
                )
                o_sb = stage_pool.tile([P, 2 * NT], F32, tag="stage", name=f"o{nt2}")
                for ck in range(NCH):
                    for dt in range(NNT):
                        nc.tensor.matmul(
                            ps[:, dt * NT : (dt + 1) * NT],
                            lhsT=aot[ck][:, nt2 * P : (nt2 + 1) * P],
                            rhs=wp_sb[:, ck * C + dt * NT : ck * C + (dt + 1) * NT],
                            start=(ck == 0),
                            stop=(ck == NCH - 1),
                        )
                # finer drain granularity on the last tile shortens the
                # add->DMA tail after the final matmul
                nq = 4 if nt2 == NMT - 1 else 2
                qw = 2 * NT // nq
                for dq in range(nq):
                    nc.vector.tensor_add(
                        o_sb[:, dq * qw : (dq + 1) * qw],
                        ps[:, dq * qw : (dq + 1) * qw],
                        bias_sb[:, dq * qw : (dq + 1) * qw],
                    )
                    nc.sync.dma_start(
                        out[nt2 * P : (nt2 + 1) * P, dq * qw : (dq + 1) * qw],
                        o_sb[:, dq * qw : (dq + 1) * qw],
                    )

        # ---------- pipeline ----------
        # pair-0 qk-proj + pair-0 S/exp run while x/wqk1/wv stream in;
        # v-blocks follow, then PV(0) consumes them and the steady loop runs.
        qks = {}
        qp0, kp0 = emit_qkproj(0)
        e00 = emit_s_exp_nt(0, 0, qp0, kp0)
        e01 = emit_s_exp_nt(0, 1, qp0, kp0)
        qks[1] = emit_qkproj(1)
        load_wqk(2)
        for mt in range(NMT):
            emit_vblock(mt)
        emit_pv_nt(0, 0, e00)
        emit_pv_nt(0, 1, e01)
        qks[2] = emit_qkproj(2)
        for p in range(1, NPAIR):
            qp, kp = qks.pop(p)
            e0 = emit_s_exp_nt(p, 0, qp, kp)
            emit_pv_nt(p, 0, e0)
            e1 = emit_s_exp_nt(p, 1, qp, kp)
            emit_pv_nt(p, 1, e1)
            if p + 2 < NPAIR:
                load_wqk(p + 2)
                qks[p + 2] = emit_qkproj(p + 2)
            if p == 6:
                load_wp()
        emit_outproj()

    nc.compile()
    _optimize_pe_stream(nc)
    return nc


def make_in_maps(x, w_qkv, w_proj, b_proj):
    import ml_dtypes

    bf16 = ml_dtypes.bfloat16
    # packed per-pair qk weights: wqkP[p*128+r, ck*128 + j]       = Wq[p,j,ck,r]
    #                             wqkP[p*128+r, C + ck*128 + j]   = Wk[p,j,ck,r]
    Wq = w_qkv[:C].reshape(NPAIR, P, NCH, P)        # [p, j, ck, r]
    Wk = w_qkv[C : 2 * C].reshape(NPAIR, P, NCH, P)
    Aq = Wq.transpose(0, 3, 2, 1)                   # [p, r, ck, j]
    Ak = Wk.transpose(0, 3, 2, 1)
    wqkP = np.ascontiguousarray(
        np.concatenate([Aq.reshape(NPAIR, P, C), Ak.reshape(NPAIR, P, C)], axis=2)
        .reshape(NPAIR * P, 2 * C)
        .astype(bf16)
    )
    wvT = np.ascontiguousarray(w_qkv[2 * C :].T.astype(bf16))
    wpT = np.ascontiguousarray(w_proj.T.astype(bf16))
    bias_bc = np.ascontiguousarray(
        np.broadcast_to(b_proj, (P, C)).astype(np.float32)
    )
    in_maps = []
    for b in range(B):
        in_maps.append(
            {
                "xT": np.ascontiguousarray(x[b].T.astype(bf16)),
                "wqkP": wqkP,
                "wvT": wvT,
                "wpT": wpT,
                "bias_bc": bias_bc,
            }
        )
    return in_maps


_CACHED_NC = None


def kernel(x, w_qkv, w_proj, b_proj):
    global _CACHED_NC
    x = np.asarray(x, dtype=np.float32)
    w_qkv = np.asarray(w_qkv, dtype=np.float32)
    w_proj = np.asarray(w_proj, dtype=np.float32)
    b_proj = np.asarray(b_proj, dtype=np.float32)
    if _CACHED_NC is None:
        _CACHED_NC = build_module()
    nc = _CACHED_NC
    in_maps = make_in_maps(x, w_qkv, w_proj, b_proj)
    res = bass_utils.run_bass_kernel_spmd(nc, in_maps, core_ids=list(range(B)))
    return np.stack([res.results[b]["out"] for b in range(B)], axis=0)


if __name__ == "__main__":
    nc = build_module()
    ninst = sum(len(b.instructions) for b in nc.m.functions[0].blocks)
    print("module built ok;", ninst, "instructions")


# revision 46
# speedup vs baseline: 1.2863x; 1.0370x over previous
"""Multi-head attention (B=8, N=1024, C=1024, H=16) on 8 Trainium2 NeuronCores.

Sharding: pure data-parallel — one batch element per core, weights replicated,
no collectives.

v3 design (vs v2): startup DMA-issue fixes (per-pair packed wqk = 1 DMA/pair,
single 3D-AP DMAs for wv/wp, ones columns via memset instead of scatter DMA,
weight loads issued from the scalar-engine HWDGE queue so the sync queue only
carries x/wqk), pipeline reordered so pair-0 S+exp runs before the v-projection
(covers the wv DMA window), and the softmax-denominator broadcast done with
gpsimd partition_broadcast + DVE multiply instead of a DRAM bounce.

Per-core algorithm:
  v-proj:    v[m, dv] natural layout, interleaved [m, 16*(64+1)] with a ones
             column per head (PV then emits softmax denominators for free).
  qk-proj:   per pair p: qp[c(2 heads), n], kp[c, m] bf16 tiles in SBUF.
  attention: per (pair, nt half):
               S^T chunks for heads A,B into [128,1024] psum pairs,
               exp (ACT, 1024-wide, scale=1/8) -> eA/eB bf16 [128, 4096]
               PV: U_aug[65, nt] = v_aug.T @ expS accumulated over 8 m-chunks
             U -> SBUF (fp32, frees psum), den row 64 -> reciprocal ->
             partition_broadcast -> DVE multiply into attn_outT[c, n] bf16.
  out-proj:  out[n, d] = attn_outT.T @ wpT + bias, fp32 out.
"""

import sys

if "/opt/trn_rl_repo" not in sys.path:
    sys.path.insert(0, "/opt/trn_rl_repo")

from contextlib import ExitStack

import numpy as np

import concourse.bass as bass
import concourse.mybir as mybir
from concourse import bacc
import concourse.tile as tile
from concourse import bass_utils

B, N, C, H = 8, 1024, 1024, 16
HD = C // H          # 64
SCALE = HD ** -0.5   # 0.125
P = 128              # SBUF partitions
NT = 512             # moving-dim tile (fp32 PSUM bank limit)
NCH = C // P         # 8 contraction chunks over channels
NMT = N // P         # 8 token tiles of 128
NNT = N // NT        # 2 token tiles of 512
NPAIR = H // 2       # 8 head pairs
F32 = mybir.dt.float32
BF16 = mybir.dt.bfloat16
EXP = mybir.ActivationFunctionType.Exp


def _wait_key(w):
    return (w.sync_type, w.id, w.wait_mode, w.wait_value)


def _weights_sig(ldw):
    a = ldw.ins[0]
    return (a.memref, a.offset, tuple(tuple(x) for x in a.ap), str(a.dtype))


def _optimize_pe_stream(nc):
    """Post-compile peephole pass over the PE instruction stream.

    Operates on the PE-only subsequence (other engines' instructions are
    interleaved in the block list but the PE sequencer only sees its own
    stream; cross-engine ordering is carried entirely by semaphores).

    Rule 1 (dedupe): a wait-free LDWEIGHTS reloading the weights already in
      the array (and already consumed by a matmul) is deleted. Wait-carrying
      LDWs are kept: waits only function on LDWEIGHTS (the PE hw-decoder
      ignores waits on MATMUL), and the first load of a compiler
      [LDW, LDW, MM, MM] prefetch pair may run before the DMA-complete wait.
    Rule 2 (hoist): [LDW_A, MM_A@(0,0) K=64, LDW_B, MM_B@(64,0) K=64] ->
      [LDW_A, LDW_B, MM_A, MM_B] so the two matmuls execute concurrently on
      disjoint row groups; only when LDW_B's waits are implied by LDW_A's
      (same semaphore, same-or-lower threshold), so the earlier wait position
      cannot deadlock.
    """
    import concourse.mybir as mybir
    from collections import deque

    n_dedupe = n_hoist = 0
    for f in nc.m.functions:
        for blk in f.blocks:
            insts = blk.instructions
            pe = [a for a in insts if getattr(a, "engine", None) == mybir.EngineType.PE]
            out = []
            i = 0
            n = len(pe)
            cur_sig = None
            cur_consumed = False
            while i < n:
                a = pe[i]
                if isinstance(a, mybir.InstLdweights):
                    sig = _weights_sig(a)
                    if (
                        sig == cur_sig
                        and cur_consumed
                        and (
                            a.sync_info is None
                            or not (a.sync_info.on_wait or a.sync_info.on_update)
                        )
                        and i + 1 < n
                        and isinstance(pe[i + 1], mybir.InstMatmult)
                    ):
                        out.append(pe[i + 1])
                        i += 2
                        n_dedupe += 1
                        cur_consumed = True
                        continue
                    if (
                        i + 3 < n
                        and isinstance(pe[i + 1], mybir.InstMatmult)
                        and isinstance(pe[i + 2], mybir.InstLdweights)
                        and isinstance(pe[i + 3], mybir.InstMatmult)
                    ):
                        ldw1, mm1, ldw2, mm2 = pe[i : i + 4]
                        tp1 = mm1.tile_position
                        tp2 = mm2.tile_position
                        if (
                            tp1 is not None
                            and tp2 is not None
                            and tuple(tp1) == (0, 0)
                            and tuple(tp2) == (64, 0)
                            and ldw1.ins[0].ap[0][1] == 64
                            and ldw2.ins[0].ap[0][1] == 64
                        ):
                            w1 = [
                                _wait_key(w)
                                for w in (
                                    ldw1.sync_info.on_wait if ldw1.sync_info else []
                                )
                            ]
                            w2 = [
                                _wait_key(w)
                                for w in (
                                    ldw2.sync_info.on_wait if ldw2.sync_info else []
                                )
                            ]
                            implied = all(
                                any(
                                    k[0] == kk[0]
                                    and k[1] == kk[1]
                                    and k[2] == kk[2]
                                    and k[3] <= kk[3]
                                    for kk in w1
                                )
                                for k in w2
                            )
                            if implied and not (
                                ldw2.sync_info and ldw2.sync_info.on_update
                            ):
                                out.extend([ldw1, ldw2, mm1, mm2])
                                cur_sig = _weights_sig(ldw2)
                                cur_consumed = True
                                i += 4
                                n_hoist += 1
                                continue
                    cur_sig = sig
                    cur_consumed = False
                elif isinstance(a, mybir.InstMatmult):
                    cur_consumed = True
                else:
                    cur_sig = None  # unknown PE instruction: be conservative
                    cur_consumed = False
                out.append(a)
                i += 1
            # weave the transformed PE stream back into the block, keeping
            # non-PE instructions in place; trailing PE slots left over from
            # deletions are simply skipped.
            pe_q = deque(out)
            new_insts = []
            for a in insts:
                if getattr(a, "engine", None) == mybir.EngineType.PE:
                    if pe_q:
                        new_insts.append(pe_q.popleft())
                else:
                    new_insts.append(a)
            assert not pe_q, "transformed PE stream longer than original slots"
            blk.instructions = new_insts
    print(f"_optimize_pe_stream: {n_dedupe} LDW deduped, {n_hoist} LDW hoisted")


def build_module():
    nc = bacc.Bacc("TRN2", target_bir_lowering=False, debug=False, num_devices=B)

    xT = nc.dram_tensor("xT", [C, N], BF16, kind="ExternalInput").ap()
    # per-pair packed qk weights: row p*128+r, col ck*256 + (q j | k j)
    wqkP = nc.dram_tensor("wqkP", [NPAIR * P, NCH * 2 * P], BF16, kind="ExternalInput").ap()
    wvT = nc.dram_tensor("wvT", [C, C], BF16, kind="ExternalInput").ap()
    wpT = nc.dram_tensor("wpT", [C, C], BF16, kind="ExternalInput").ap()
    bias = nc.dram_tensor("bias_bc", [P, C], F32, kind="ExternalInput").ap()
    out = nc.dram_tensor("out", [N, C], F32, kind="ExternalOutput").ap()

    with tile.TileContext(nc) as tc, ExitStack() as ctx:
        dram = ctx.enter_context(tc.tile_pool(name="dram", bufs=1, space="DRAM"))
        rden_d = dram.tile([H * NNT, NT], F32, tag="rden_d", name="rden_d")

        xt_pool = ctx.enter_context(tc.tile_pool(name="xt", bufs=8))
        qk_pool = ctx.enter_context(tc.tile_pool(name="qk", bufs=7))
        v_pool = ctx.enter_context(tc.tile_pool(name="v", bufs=8))
        e_pool = ctx.enter_context(tc.tile_pool(name="e", bufs=6))
        u_pool = ctx.enter_context(tc.tile_pool(name="u", bufs=6))
        aot_pool = ctx.enter_context(tc.tile_pool(name="aot", bufs=1))
        w_pool = ctx.enter_context(tc.tile_pool(name="wst", bufs=2))
        wqk_pool = ctx.enter_context(tc.tile_pool(name="wqk", bufs=4))
        rbc_pool = ctx.enter_context(tc.tile_pool(name="rbc", bufs=6))
        one_pool = ctx.enter_context(tc.tile_pool(name="one", bufs=1))
        stage_pool = ctx.enter_context(tc.tile_pool(name="stage", bufs=2))
        s_psum = ctx.enter_context(tc.tile_pool(name="s_ps", bufs=2, space="PSUM"))
        pv_psum = ctx.enter_context(tc.tile_pool(name="pv_ps", bufs=2, space="PSUM"))
        pj_psum = ctx.enter_context(tc.tile_pool(name="pj_ps", bufs=1, space="PSUM"))

        # ---------- qk weight load (packed per pair; 2 dmas each) ----------
        wqk_tiles = {}

        def load_wqk(p, eng=None):
            eng = eng or nc.sync
            w_t = wqk_pool.tile([P, NCH * 2 * P], BF16, tag="wqk", name=f"wqk{p}")
            # q half then k half so the first q matmul doesn't wait for k
            eng.dma_start(w_t[:, 0:C], wqkP[p * P : (p + 1) * P, 0:C])
            eng.dma_start(w_t[:, C : 2 * C], wqkP[p * P : (p + 1) * P, C : 2 * C])
            wqk_tiles[p] = w_t

        # ---------- input loads: x on sync queue, weights on scalar queue ----
        # few, large dma_starts: the DMA-semaphore ring only allows ~8-10
        # outstanding dma_starts, so a long run of small loads stalls the
        # issue queue for tens of us.
        # x per-chunk tiles, interleaved across both HWDGE queues so the 2MB
        # streams at full aggregate bandwidth; wqk0 q first (first MM dep),
        # wqk0 k and wqk1 woven in so each lands just before its matmuls.
        w0 = wqk_pool.tile([P, NCH * 2 * P], BF16, tag="wqk", name="wqk0")
        w1 = wqk_pool.tile([P, NCH * 2 * P], BF16, tag="wqk", name="wqk1")
        xts = [xt_pool.tile([P, N], BF16, tag="xt", name=f"xt{t}") for t in range(NCH)]
        nc.sync.dma_start(w0[:, 0:C], wqkP[0:P, 0:C])
        for t in range(0, NCH, 2):
            nc.sync.dma_start(xts[t], xT[t * P : (t + 1) * P, :])
        nc.sync.dma_start(w0[:, C : 2 * C], wqkP[0:P, C : 2 * C])
        nc.sync.dma_start(w1[:, 0:C], wqkP[P : 2 * P, 0:C])
        nc.sync.dma_start(w1[:, C : 2 * C], wqkP[P : 2 * P, C : 2 * C])
        for t in range(1, NCH, 2):
            nc.scalar.dma_start(xts[t], xT[t * P : (t + 1) * P, :])
        wqk_tiles[0] = w0
        wqk_tiles[1] = w1

        # wv as one [128, 8*1024] tile: block ck at cols [ck*1024, +1024)
        wv_sb = w_pool.tile([P, NCH * C], BF16, tag="wst", name="wv_sb")
        for h in range(2):
            wv_src = bass.AP(
                tensor=wvT.tensor,
                offset=h * 4 * P * C,
                ap=[[C, P], [P * C, 4], [1, C]],
            )
            nc.scalar.dma_start(wv_sb[:, h * 4 * C : (h + 1) * 4 * C], wv_src)

        vsb = []
        for mt in range(NMT):
            v_t = v_pool.tile([P, H * (HD + 1)], BF16, tag="v", name=f"v{mt}")
            # contiguous full-tile fill; the v-proj copy then overwrites the
            # value columns, leaving 1.0 in each head's 65th (ones) column
            nc.gpsimd.memset(v_t, 1.0)
            vsb.append(v_t)
        aot = [
            aot_pool.tile([P, N], BF16, tag=f"aot{t}", name=f"aot{t}")
            for t in range(NCH)
        ]

        # ---------- v projection (natural layout + ones cols) ----------
        def emit_vblock(mt):
                # the s pool is idle during the v-block phase (pair-0 S/exp
                # finished, pair-1 S starts after PV(0)); alternating pools
                # double-buffers the v-proj psum drains
                pool = pj_psum if mt % 2 == 0 else s_psum
                ps = pool.tile(
                    [P, 2 * NT], F32, tag="pj" if mt % 2 == 0 else "s",
                    name=f"psv{mt}",
                )
                for ck in range(NCH):
                    for dvt in range(NNT):
                        nc.tensor.matmul(
                            ps[:, dvt * NT : (dvt + 1) * NT],
                            lhsT=xts[ck][:, mt * P : (mt + 1) * P],
                            rhs=wv_sb[:, ck * C + dvt * NT : ck * C + (dvt + 1) * NT],
                            start=(ck == 0),
                            stop=(ck == NCH - 1),
                        )
                for dvt in range(NNT):
                    nc.vector.tensor_copy(
                        vsb[mt].rearrange("p (h w) -> p h w", w=HD + 1)[
                            :, dvt * NCH : (dvt + 1) * NCH, 0:HD
                        ],
                        ps[:, dvt * NT : (dvt + 1) * NT].rearrange(
                            "p (h w) -> p h w", w=HD
                        ),
                    )

        # ---------- qk projection for one head pair, SBUF-resident ----------
        def emit_qkproj(p):
            qp = qk_pool.tile([P, N], BF16, tag="qk", name=f"qp{p}")
            kp = qk_pool.tile([P, N], BF16, tag="qk", name=f"kp{p}")
            w_t = wqk_tiles.pop(p)
            for which, dstt in ((0, qp), (1, kp)):
                ps = pj_psum.tile([P, 2 * NT], F32, tag="pj", name=f"psqk{which}_{p}")
                for ck in range(NCH):
                    for nt_ in range(NNT):
                        nc.tensor.matmul(
                            ps[:, nt_ * NT : (nt_ + 1) * NT],
                            lhsT=w_t[:, which * C + ck * P : which * C + (ck + 1) * P],
                            rhs=xts[ck][:, nt_ * NT : (nt_ + 1) * NT],
                            start=(ck == 0),
                            stop=(ck == NCH - 1),
                        )
                for nt_ in range(NNT):
                    nc.vector.tensor_copy(
                        dstt[:, nt_ * NT : (nt_ + 1) * NT],
                        ps[:, nt_ * NT : (nt_ + 1) * NT],
                    )
            return qp, kp

        # ---------- attention ----------
        pair_units = {}

        def emit_denorm(p, nt_, punits):
            """den rows -> DRAM bounce -> partition-broadcast read ->
            reciprocal -> DVE multiply into the attn-out tiles (bf16 cast on
            write). The reciprocal runs AFTER the DMA round-trip so the
            exposed chain is one SBUF->DRAM->SBUF hop plus two DVE ops.
            Runs per (pair, nt-half) so the out-proj's first n-half unblocks
            as soon as the last pair's nt=0 units are normalized."""
            g = p * 4 + nt_ * 2
            for i, (h, u_t) in enumerate(punits):
                nc.sync.dma_start(rden_d[g + i : g + i + 1, :], u_t[HD : HD + 1, :])
            for i, (h, u_t) in enumerate(punits):
                rbc = rbc_pool.tile([HD, NT], F32, tag="rbc", name=f"rbc{h}_{nt_}")
                src_ = rden_d[g + i : g + i + 1, :]
                bsrc = bass.AP(
                    tensor=src_.tensor,
                    offset=src_.offset,
                    ap=[[0, HD], list(src_.ap[-1])],
                )
                nc.sync.dma_start(out=rbc, in_=bsrc)
                rr = rbc_pool.tile([HD, NT], F32, tag="rbc", name=f"rr{h}_{nt_}")
                nc.vector.reciprocal_approx_fast(out=rr, in_=rbc)
                ct, prow = h // 2, (h % 2) * HD
                nc.vector.tensor_mul(
                    aot[ct][prow : prow + HD, nt_ * NT : (nt_ + 1) * NT],
                    u_t[0:HD, :],
                    rr,
                )

        def emit_s_exp_nt(p, nt_, qp, kp):
            """S^T + exp for one (pair, n-half). Heads A and B share one
            [128, 1024] psum tile per m-chunk (A in the low bank, B in the
            high bank) so both matmuls become ready together; the post-compile
            pass hoists B's LDWEIGHTS above A's matmul, making the two K=64
            matmuls (array rows 0-63 / 64-127) run concurrently.
            Returns two e tiles [128, 4096] (mc 0-3 and mc 4-7), each laid
            out as [A_mc|B_mc|...]; the split lets PV release the first half
            mid-chain so the next pair's S can reuse the slots earlier."""
            eA = e_pool.tile([P, NMT * NT], BF16, tag="e", name=f"e{p}_{nt_}a")
            eB = e_pool.tile([P, NMT * NT], BF16, tag="e", name=f"e{p}_{nt_}b")
            for mc in range(NMT):
                e_t = eA if mc < 4 else eB
                s_t = s_psum.tile([P, 2 * NT], F32, tag="s", name=f"s{p}_{nt_}_{mc}")
                # high priority: the S pair feeds ACT (the attention-phase
                # pacer) and must pop back-to-back so the post-compile hoist
                # can make the two K=64 row-tiles run concurrently.
                with tc.high_priority():
                    nc.tensor.matmul(
                        s_t[:, 0:NT],
                        lhsT=kp[0:HD, mc * P : (mc + 1) * P],
                        rhs=qp[0:HD, nt_ * NT : (nt_ + 1) * NT],
                        start=True,
                        stop=True,
                    )
                    nc.tensor.matmul(
                        s_t[:, NT : 2 * NT],
                        lhsT=kp[HD:P, mc * P : (mc + 1) * P],
                        rhs=qp[HD:P, nt_ * NT : (nt_ + 1) * NT],
                        start=True,
                        stop=True,
                    )
                nc.scalar.activation(
                    e_t[:, (mc % 4) * 2 * NT : ((mc % 4) + 1) * 2 * NT],
                    s_t,
                    EXP,
                    scale=SCALE,
                )
            return eA, eB

        def emit_pv_nt(p, nt_, e_h):
            """PV for BOTH heads of the pair over one n-half, the two chains
            interleaved per m-chunk in the two pv psum slots. Both heads pass
            mc 0-3 together, so the first e-half frees as early as possible
            for the next pair's S chain."""
            eA, eB = e_h
            hA, hB = 2 * p, 2 * p + 1
            psA = pv_psum.tile([HD + 1, NT], F32, tag="pv", name=f"pu{hA}_{nt_}")
            psB = pv_psum.tile([HD + 1, NT], F32, tag="pv", name=f"pu{hB}_{nt_}")
            for mc in range(NMT):
                e_t = eA if mc < 4 else eB
                for ps, j in ((psA, 0), (psB, 1)):
                    nc.tensor.matmul(
                        ps,
                        lhsT=vsb[mc][:, (2 * p + j) * (HD + 1) : (2 * p + j + 1) * (HD + 1)],
                        rhs=e_t[:, ((mc % 4) * 2 + j) * NT : ((mc % 4) * 2 + j + 1) * NT],
                        start=(mc == 0),
                        stop=(mc == NMT - 1),
                    )
            units = []
            for h, ps in ((hA, psA), (hB, psB)):
                u_t = u_pool.tile([HD + 1, NT], F32, tag="u", name=f"u{h}_{nt_}")
                nc.vector.tensor_copy(u_t, ps)
                units.append((h, u_t))
            emit_denorm(p, nt_, units)

        # ---------- output projection + bias ----------
        # dt halves paired on the stationary aot chunk, single [128, 1024]
        # psum per n-tile; alternate between the pj and s psum pools (the s
        # pool is free by the tail) to keep the tail double-buffered.
        wp_holder = []
        bias_holder = []

        def load_wp():
            bias_sb = one_pool.tile([P, C], F32, tag="bias", name="bias_sb")
            nc.scalar.dma_start(bias_sb, bias)
            bias_holder.append(bias_sb)
            wp_sb = w_pool.tile([P, NCH * C], BF16, tag="wst", name="wp_sb")
            wp_src = bass.AP(
                tensor=wpT.tensor,
                offset=0,
                ap=[[C, P], [P * C, NCH], [1, C]],
            )
            nc.scalar.dma_start(wp_sb, wp_src)
            wp_holder.append(wp_sb)

        def emit_outproj():
            bias_sb = bias_holder[0]
            wp_sb = wp_holder[0]
            for nt2 in range(NMT):
                pool = pj_psum if nt2 % 2 == 0 else s_psum
                ps = pool.tile(
                    [P, 2 * NT], F32, tag="pj" if nt2 % 2 == 0 else "s",
                    name=f"pso{nt2}",
                )
                o_sb = stage_pool.tile([P, 2 * NT], F32, tag="stage", name=f"o{nt2}")
                for ck in range(NCH):
                    for dt in range(NNT):
                        nc.tensor.matmul(
                            ps[:, dt * NT : (dt + 1) * NT],
                            lhsT=aot[ck][:, nt2 * P : (nt2 + 1) * P],
                            rhs=wp_sb[:, ck * C + dt * NT : ck * C + (dt + 1) * NT],
                            start=(ck == 0),
                            stop=(ck == NCH - 1),
                        )
                # finer drain granularity on the last tile shortens the
                # add->DMA tail after the final matmul
                nq = 4 if nt2 == NMT - 1 else 2
                qw = 2 * NT // nq
                for dq in range(nq):
                    nc.vector.tensor_add(
                        o_sb[:, dq * qw : (dq + 1) * qw],
                        ps[:, dq * qw : (dq + 1) * qw],
                        bias_sb[:, dq * qw : (dq + 1) * qw],
                    )
                    nc.sync.dma_start(
                        out[nt2 * P : (nt2 + 1) * P, dq * qw : (dq + 1) * qw],
                        o_sb[:, dq * qw : (dq + 1) * qw],
                    )

        # ---------- pipeline ----------
        # pair-0 qk-proj + pair-0 S/exp run while x/wqk1/wv stream in;
        # v-blocks follow, then PV(0) consumes them and the steady loop runs.
        qks = {}
        qp0, kp0 = emit_qkproj(0)
        e00 = emit_s_exp_nt(0, 0, qp0, kp0)
        e01 = emit_s_exp_nt(0, 1, qp0, kp0)
        qks[1] = emit_qkproj(1)
        load_wqk(2)
        for mt in range(NMT):
            emit_vblock(mt)
        emit_pv_nt(0, 0, e00)
        emit_pv_nt(0, 1, e01)
        qks[2] = emit_qkproj(2)
        for p in range(1, NPAIR):
            qp, kp = qks.pop(p)
            e0 = emit_s_exp_nt(p, 0, qp, kp)
            emit_pv_nt(p, 0, e0)
            e1 = emit_s_exp_nt(p, 1, qp, kp)
            emit_pv_nt(p, 1, e1)
            if p + 2 < NPAIR:
                load_wqk(p + 2)
                qks[p + 2] = emit_qkproj(p + 2)
            if p == 6:
                load_wp()
        emit_outproj()

    nc.compile()
    _optimize_pe_stream(nc)
    return nc


def make_in_maps(x, w_qkv, w_proj, b_proj):
    import ml_dtypes

    bf16 = ml_dtypes.bfloat16
    # packed per-pair qk weights: wqkP[p*128+r, ck*128 + j]       = Wq[p,j,ck,r]
    #                             wqkP[p*128+r, C + ck*128 + j]   = Wk[p,j,ck,r]
    Wq = w_qkv[:C].reshape(NPAIR, P, NCH, P)        # [p, j, ck, r]
    Wk = w_qkv[C : 2 * C].reshape(NPAIR, P, NCH, P)
    Aq = Wq.transpose(0, 3, 2, 1)                   # [p, r, ck, j]
    Ak = Wk.transpose(0, 3, 2, 1)
    wqkP = np.ascontiguousarray(
        np.concatenate([Aq.reshape(NPAIR, P, C), Ak.reshape(NPAIR, P, C)], axis=2)
        .reshape(NPAIR * P, 2 * C)
        .astype(bf16)
    )
    wvT = np.ascontiguousarray(w_qkv[2 * C :].T.astype(bf16))
    wpT = np.ascontiguousarray(w_proj.T.astype(bf16))
    bias_bc = np.ascontiguousarray(
        np.broadcast_to(b_proj, (P, C)).astype(np.float32)
    )
    in_maps = []
    for b in range(B):
        in_maps.append(
            {
                "xT": np.ascontiguousarray(x[b].T.astype(bf16)),
                "wqkP": wqkP,
                "wvT": wvT,
                "wpT": wpT,
                "bias_bc": bias_bc,
            }
        )
    return in_maps


_CACHED_NC = None


def kernel(x, w_qkv, w_proj, b_proj):
    global _CACHED_NC
    x = np.asarray(x, dtype=np.float32)
    w_qkv = np.asarray(w_qkv, dtype=np.float32)
    w_proj = np.asarray(w_proj, dtype=np.float32)
    b_proj = np.asarray(b_proj, dtype=np.float32)
    if _CACHED_NC is None:
        _CACHED_NC = build_module()
    nc = _CACHED_NC
    in_maps = make_in_maps(x, w_qkv, w_proj, b_proj)
    res = bass_utils.run_bass_kernel_spmd(nc, in_maps, core_ids=list(range(B)))
    return np.stack([res.results[b]["out"] for b in range(B)], axis=0)


if __name__ == "__main__":
    nc = build_module()
    ninst = sum(len(b.instructions) for b in nc.m.functions[0].blocks)
    print("module built ok;", ninst, "instructions")
